# revision 2
# baseline (speedup 1.0000x reference)
"""Head-parallel multi-head attention on 8 Trainium2 NeuronCores (v3).

Sharding: 2 heads per core (head axis split across 8 cores). Each core
computes its heads' Q/K/V projections (block-diagonal 128x128 weights,
both heads packed on the partition axis), full attention for its 2
heads, and a per-head partial W_o projection over its 128 head-dims.
The host sums the 8 partial outputs (the all-gather + W_o is
algebraically a sum of per-core partial matmuls) and adds b_o.

v3 over v2 (~229us): kt-lag pipeline instead of phase-lag.
  * PV(p, kt) runs 2 k-tiles behind exp(p, kt) inside the SAME phase
    (v2 lagged a full phase), and normalize/outproj/DMA for phase p
    flush during phase p+1 (v2: p+2).  The drain tail after the last
    exp shrinks from ~42us to a few us.
  * startup: x is DMA'd in 512-col chunks and Q/K are produced in
    512-col quarter matmuls, so the first scores matmul only waits for
    one small DMA chain; first exp fires ~3us in (v2: ~16us).
  * softmax 1/denom comes from a single DVE reciprocal (bf16 out)
    instead of the two-ACTIVATE exp(-ln d) chain: ACT runs only the
    128 [128,1024] exps (its hard floor, ~1.11us each).
  * per-phase PSUM stays at exactly 8 banks: scores ping-pong
    2x[128,1024] + oa(p)/oa(p-1) 2x[128,1024]; outproj(p-1) writes
    into retired oa(p-1) slices, late QKV parts reuse retired banks.
"""

import os
import sys
from contextlib import ExitStack

import numpy as np

for _p in ("/opt/trn_rl_repo", os.path.expanduser("~/.axon_site/_ro/trn_rl_repo")):
    if os.path.isdir(_p) and _p not in sys.path:
        sys.path.append(_p)

import ml_dtypes

import concourse.bass as bass
import concourse.tile as tile
from concourse import mybir
from concourse.bass_utils import run_bass_kernel_spmd

B, S, E, H = 2, 2048, 1024, 16
Dh = E // H           # 64
NCORES = 8
HPC = H // NCORES     # 2 heads per core
PD = HPC * Dh         # 128 pair dims per core
QC = 512              # q-chunk width
NQC = S // QC         # 4
KT = 128              # k-tile rows
NKT = S // KT         # 16
F32 = mybir.dt.float32
BF16 = mybir.dt.bfloat16
EXP = mybir.ActivationFunctionType.Exp
LN = mybir.ActivationFunctionType.Ln
BF = ml_dtypes.bfloat16

# 1/denom on DVE (one InstReciprocal) vs ACT exp(-ln d) fallback
USE_DVE_RECIP = True


def split_multi_waits(nc):
    """Split multi-wait instructions into chained single-wait EventSemaphores.

    The walrus build here accepts at most ONE sync-wait command per
    instruction, while Tile emits several. Rewrite each instruction with
    N>1 waits into (N-1) same-engine EventSemaphore instructions (one
    wait each) followed by the instruction keeping its last wait --
    per-engine program order makes this equivalent.
    """
    n_split = 0
    for f in nc.m.functions:
        for blk in f.blocks:
            insts = list(blk.instructions)
            new = []
            for inst in insts:
                si = inst.sync_info
                waits = list(si.on_wait) if si is not None and si.on_wait else []
                if len(waits) > 1:
                    for j, w in enumerate(waits[:-1]):
                        ev = mybir.InstEventSemaphore(
                            name=f"{inst.name}-wsplit{j}", ins=[], outs=[]
                        )
                        ev.engine = inst.engine
                        ev.sync_info = mybir.SyncInfo(on_wait=[w], on_update=[])
                        nc.register_instruction(ev, overwrite=True)
                        new.append(ev)
                    si.on_wait = waits[-1:]
                    n_split += 1
                new.append(inst)
            blk.instructions = new
    return n_split


def build_program():
    nc = bass.Bass("TRN2", target_bir_lowering=False, debug=False)

    xtb = nc.dram_tensor("xtb", [B, PD, S], BF16, kind="ExternalInput").ap()
    wqkv = nc.dram_tensor("wqkv", [3, PD, PD], BF16, kind="ExternalInput").ap()
    bqk = nc.dram_tensor("bqk", [2, PD, 1], F32, kind="ExternalInput").ap()
    bvb8 = nc.dram_tensor("bvb8", [PD, 8 * PD], F32, kind="ExternalInput").ap()
    wo2 = nc.dram_tensor("wo2", [HPC, Dh, E], BF16, kind="ExternalInput").ap()
    out = nc.dram_tensor("out", [B, S, E], BF16, kind="ExternalOutput").ap()

    with tile.TileContext(nc) as tc, ExitStack() as ctx:
        const = ctx.enter_context(tc.tile_pool(name="const", bufs=1))
        perb = ctx.enter_context(tc.tile_pool(name="perb", bufs=2))
        slabp = ctx.enter_context(tc.tile_pool(name="slab", bufs=6))
        normp = ctx.enter_context(tc.tile_pool(name="norm", bufs=2))
        outp = ctx.enter_context(tc.tile_pool(name="outp", bufs=4))
        ps = ctx.enter_context(tc.tile_pool(name="ps", bufs=2, space="PSUM"))

        # ---- constants: order so the first Q/K quarter's deps land first ----
        w_sb = []
        for i in range(3):
            w_sb.append(const.tile([PD, PD], BF16, tag=f"w{i}", name=f"w{i}"))
        nc.gpsimd.dma_start(out=w_sb[0][:], in_=wqkv[0])
        nc.gpsimd.dma_start(out=w_sb[1][:], in_=wqkv[1])
        xtb_sb = const.tile([PD, B, S], BF16)
        nc.sync.dma_start(out=xtb_sb[:, 0, 0:QC], in_=xtb[0, :, 0:QC])
        bq_sb = const.tile([PD, 1], F32, tag="bq")
        nc.sync.dma_start(out=bq_sb[:], in_=bqk[0])
        bk_sb = const.tile([PD, 1], F32, tag="bk")
        nc.sync.dma_start(out=bk_sb[:], in_=bqk[1])
        nc.gpsimd.dma_start(out=w_sb[2][:], in_=wqkv[2])
        bvb8_sb = const.tile([PD, 8 * PD], F32, tag="bvb8")
        nc.sync.dma_start(out=bvb8_sb[:], in_=bvb8)
        for g in range(1, 4):
            nc.sync.dma_start(out=xtb_sb[:, 0, g * QC:(g + 1) * QC],
                              in_=xtb[0, :, g * QC:(g + 1) * QC])
        nc.gpsimd.dma_start(out=xtb_sb[:, 1, :], in_=xtb[1])
        wop_sb = const.tile([PD, E], BF16, tag="wop")
        for h in range(HPC):
            nc.gpsimd.dma_start(out=wop_sb[h * Dh:(h + 1) * Dh, :], in_=wo2[h])
        ones_sb = const.tile([1, Dh], BF16, tag="ones")
        nc.vector.memset(ones_sb[:], 1.0)

        # ---- pipeline state ----
        phases = [(b, c) for b in range(B) for c in range(NQC)]
        NP = len(phases)
        slabs = {}   # (pi, kt) -> slab tile
        oas = {}     # phase idx -> oa PSUM tile [128, 1024] (rows 0:65 used)
        qts = {}     # batch -> qt tile
        kts = {}     # batch -> kt tile
        vaugs = {}   # batch -> vaug tile

        def emit_qkv_alloc(b):
            qt = perb.tile([PD, S], BF16, tag="qt", name=f"qt{b}")
            kt_t = perb.tile([PD, S], BF16, tag="kt", name=f"kt{b}")
            vaug = perb.tile([PD, NKT, HPC, Dh + 1], BF16, tag="vaug",
                             name=f"vaug{b}")
            nc.vector.memset(vaug[:, :, :, Dh], 1.0)
            qts[b], kts[b], vaugs[b] = qt, kt_t, vaug

        def qkv_quarter(b, kind, g, p=None):
            """One 512-wide Q (kind 0) or K (kind 1) quarter for batch b."""
            if p is None:
                p = ps.tile([PD, 2 * QC], F32, tag="scs", name="qkvps")
            dst = qts[b] if kind == 0 else kts[b]
            bias = bq_sb if kind == 0 else bk_sb
            sl_ = slice(g * QC, (g + 1) * QC)
            nc.tensor.matmul(p[:, 0:QC], lhsT=w_sb[kind][:],
                             rhs=xtb_sb[:, b, sl_])
            nc.vector.tensor_scalar_add(dst[:, sl_], p[:, 0:QC], bias[:])

        def qkv_half(b, kind, g, p=None):
            """One 1024-wide Q/K half (kind 0/1) or V half (kind 2)."""
            if p is None:
                p = ps.tile([PD, 2 * QC], F32, tag="scs", name="qkvps")
            if kind < 2:
                dst = qts[b] if kind == 0 else kts[b]
                bias = bq_sb if kind == 0 else bk_sb
                for j in range(2):
                    sl_ = slice((2 * g + j) * QC, (2 * g + j + 1) * QC)
                    nc.tensor.matmul(p[:, j * QC:(j + 1) * QC],
                                     lhsT=w_sb[kind][:],
                                     rhs=xtb_sb[:, b, sl_])
                nc.vector.tensor_scalar_add(
                    dst[:, 2 * g * QC:(2 * g + 2) * QC], p[:], bias[:])
            else:
                for i in range(8):
                    st = 8 * g + i
                    nc.tensor.matmul(p[:, i * PD:(i + 1) * PD],
                                     lhsT=xtb_sb[:, b, st * KT:(st + 1) * KT],
                                     rhs=w_sb[2][:])
                nc.vector.tensor_add(
                    vaugs[b][:, 8 * g:8 * (g + 1), :, 0:Dh],
                    p[:].rearrange("p (t h d) -> p t h d", t=8, h=HPC),
                    bvb8_sb[:].rearrange("p (t h d) -> p t h d", t=8, h=HPC),
                )

        def v_eighth(b, e, p=None):
            """V projection for k-tiles 4e..4e+3 (512 cols of x)."""
            if p is None:
                p = ps.tile([PD, 2 * QC], F32, tag="scs", name="qkvps")
            for i in range(4):
                st = 4 * e + i
                nc.tensor.matmul(p[:, i * PD:(i + 1) * PD],
                                 lhsT=xtb_sb[:, b, st * KT:(st + 1) * KT],
                                 rhs=w_sb[2][:])
            nc.vector.tensor_add(
                vaugs[b][:, 4 * e:4 * (e + 1), :, 0:Dh],
                p[:, 0:4 * PD].rearrange("p (t h d) -> p t h d", t=4, h=HPC),
                bvb8_sb[:, 0:4 * PD].rearrange("p (t h d) -> p t h d", t=4, h=HPC),
            )

        def emit_pv(pi, kt):
            oa = oas[pi]
            sl = slabs.pop((pi, kt))
            for h in range(HPC):
                nc.tensor.matmul(
                    oa[0:Dh + 1, h * QC:(h + 1) * QC],
                    lhsT=vaugs[phases[pi][0]][:, kt, h, :],
                    rhs=sl[:, h * QC:(h + 1) * QC],
                    start=(kt == 0), stop=(kt == NKT - 1),
                )

        def emit_recip(pi):
            """1/denom for phase pi: oa row 64 -> rr [1, 1024] bf16."""
            oa = oas[pi]
            rr = normp.tile([1, 2 * QC], BF16, tag="rr", name="rr")
            if USE_DVE_RECIP:
                with nc.allow_low_precision(reason="bf16 1/denom"):
                    nc.vector.reciprocal(rr[:], oa[Dh:Dh + 1, :])
            else:
                lnd = normp.tile([1, 2 * QC], F32, tag="lnd", name="lnd")
                nc.scalar.activation(lnd[:], oa[Dh:Dh + 1, :], LN)
                nc.scalar.activation(rr[:], lnd[:], EXP, scale=-1.0)
            return rr

        def emit_bcast(pi, rr):
            """Broadcast 1/denom across 64 partitions into oa rows 64:128."""
            oa = oas[pi]
            oa64 = oa[Dh:Dh + Dh, :]
            for h in range(HPC):
                nc.tensor.matmul(oa64[:, h * QC:(h + 1) * QC],
                                 lhsT=ones_sb[:],
                                 rhs=rr[:, h * QC:(h + 1) * QC],
                                 tile_position=(0, Dh))

        def emit_otp(pi):
            """Per-head normalized output: otp [128, 512] bf16 (h1 -> rows 64+)."""
            oa = oas[pi]
            oa64 = oa[Dh:Dh + Dh, :]
            bc = normp.tile([Dh, 2 * QC], F32, tag="bc", name="bc")
            nc.vector.tensor_copy(bc[:], oa64[:])
            otp = normp.tile([PD, QC], BF16, tag="otp", name="otp")
            for h in range(HPC):
                nc.vector.tensor_mul(otp[h * Dh:(h + 1) * Dh, :],
                                     oa[0:Dh, h * QC:(h + 1) * QC],
                                     bc[:, h * QC:(h + 1) * QC])
            return otp

        def emit_outproj_pair(pi, otp, i, outsb, slices=None):
            """Out-projection pair #i (stile i//2, echunk i%2) for phase pi."""
            oa = oas[pi]
            st, ec = i // 2, i % 2
            esl = slice(ec * QC, (ec + 1) * QC)
            if slices is None:
                sl_ = oa[:, ec * QC:(ec + 1) * QC]
            else:
                sl_ = slices[i % len(slices)]
            nc.tensor.matmul(sl_, lhsT=otp[:, st * KT:(st + 1) * KT],
                             rhs=wop_sb[:, esl])
            if slices is not None and i % 2 == 1:
                # tail only: ACT is idle after the last exp; split staging
                nc.scalar.copy(outsb[:, esl], sl_)
            else:
                nc.vector.tensor_copy(outsb[:, esl], sl_)

        def flush(fp, slot, state):
            """Flush work for phase fp, scheduled at kt position `slot`
            of the following phase (or compressed in the tail)."""
            if fp < 0:
                return
            if slot == 3:
                state["rr"] = emit_recip(fp)
            elif slot == 4:
                emit_bcast(fp, state["rr"])
            elif slot == 5:
                state["ots"] = emit_otp(fp)
            elif 6 <= slot < 14:
                i = slot - 6
                b2, c2 = phases[fp]
                st, ec = i // 2, i % 2
                if ec == 0:
                    state["outsb"] = outp.tile([KT, E], BF16, tag="outsb",
                                               name="outsb")
                emit_outproj_pair(fp, state["ots"], i, state["outsb"],
                                  slices=state.get("slices"))
                ssl = slice(c2 * QC + st * KT, c2 * QC + (st + 1) * KT)
                if state.get("slices") is not None:
                    # tail: DMA each echunk half as its copy lands
                    esl = slice(ec * QC, (ec + 1) * QC)
                    eng = (nc.sync, nc.gpsimd, nc.scalar)[i % 3]
                    eng.dma_start(out=out[b2, ssl, esl],
                                  in_=state["outsb"][:, esl])
                elif ec == 1:
                    eng = nc.sync if st % 2 == 0 else nc.gpsimd
                    eng.dma_start(out=out[b2, ssl, :], in_=state["outsb"][:])

        # QKV extras: (pi, kt) -> emit fn.  Phase 0 builds batch 0 piecewise;
        # batch 1 parts ride retired PSUM slots in phases (0,1)-(0,3).
        extras = {
            (0, 0): lambda: v_eighth(0, 0),
            (0, 1): lambda: qkv_quarter(0, 1, 1),
            (0, 2): lambda: v_eighth(0, 1),
            (0, 3): lambda: qkv_quarter(0, 1, 2),
            (0, 4): lambda: v_eighth(0, 2),
            (0, 5): lambda: qkv_quarter(0, 1, 3),
            (0, 6): lambda: v_eighth(0, 3),
            (0, 8): lambda: qkv_quarter(0, 0, 1),
            (0, 10): lambda: qkv_quarter(0, 0, 2),
            (0, 12): lambda: qkv_quarter(0, 0, 3),
            (1, 14): lambda: qkv_half(1, 1, 0),
            (1, 15): lambda: qkv_half(1, 0, 0),
            (2, 14): lambda: qkv_half(1, 1, 1, p=oas[1]),
            (2, 15): lambda: qkv_half(1, 0, 1, p=oas[1]),
            (3, 14): lambda: qkv_half(1, 2, 0, p=oas[2]),
            (3, 15): lambda: qkv_half(1, 2, 1, p=oas[2]),
        }

        state = {}
        emit_qkv_alloc(0)
        emit_qkv_alloc(1)
        qkv_quarter(0, 0, 0)   # Q chunk 0
        qkv_quarter(0, 1, 0)   # K k-tiles 0-3
        for pi, (b, c) in enumerate(phases):
            qt, kt_t = qts[b], kts[b]
            csl = slice(c * QC, (c + 1) * QC)
            oas[pi] = ps.tile([PD, 2 * QC], F32, tag="oa", name=f"oa{pi}")
            for kt in range(NKT):
                scs = ps.tile([PD, 2 * QC], F32, tag="scs", name="scs")
                for h in range(HPC):
                    hsl = slice(Dh * h, Dh * (h + 1))
                    # 2x row tiling: both heads stream concurrently
                    nc.tensor.matmul(
                        scs[:, h * QC:(h + 1) * QC],
                        lhsT=kt_t[hsl, kt * KT:(kt + 1) * KT],
                        rhs=qt[hsl, csl],
                        tile_position=(Dh * h, 0),
                    )
                sl_t = slabp.tile([PD, 2 * QC], BF16, tag="slab", name="slab")
                nc.scalar.activation(sl_t[:], scs[:], EXP, scale=0.125)
                slabs[(pi, kt)] = sl_t
                # PV: 2 k-tiles behind exp; first two slots finish pi-1
                if kt >= 2:
                    emit_pv(pi, kt - 2)
                elif pi >= 1:
                    emit_pv(pi - 1, NKT - 2 + kt)
                if pi >= 1:
                    flush(pi - 1, kt, state)
                ex = extras.get((pi, kt))
                if ex is not None:
                    ex()

        # ---- tail: finish PV for the last phase, flush it compressed ----
        last = NP - 1
        emit_pv(last, NKT - 2)
        emit_pv(last, NKT - 1)
        state["rr"] = emit_recip(last)
        emit_bcast(last, state["rr"])
        state["ots"] = emit_otp(last)
        # rotate through FOUR retired PSUM slices (oa last + last-1) so the
        # matmul->copy->DMA chain pipelines 4 deep
        state["slices"] = [
            oas[last][:, 0:QC], oas[last][:, QC:2 * QC],
            oas[last - 1][:, 0:QC], oas[last - 1][:, QC:2 * QC],
        ]
        for slot in range(6, 14):
            flush(last, slot, state)

    from concourse.library_overlay import lower_extended_insts

    lower_extended_insts(nc)
    split_multi_waits(nc)
    return nc


def prep_core_inputs(c, x, Wq, Wk, Wv, bq, bk, bv, Wo):
    h0, h1 = HPC * c, HPC * c + 1
    xT_c = np.ascontiguousarray(
        np.transpose(x[:, :, c * PD:(c + 1) * PD], (0, 2, 1))
    ).astype(BF)
    wqkv = np.zeros((3, PD, PD), np.float32)
    for i, W in enumerate((Wq, Wk, Wv)):
        wqkv[i, :Dh, :Dh] = W[h0]
        wqkv[i, Dh:, Dh:] = W[h1]
    bqk = np.stack([
        np.concatenate([bq[h0], bq[h1]])[:, None],
        np.concatenate([bk[h0], bk[h1]])[:, None],
    ]).astype(np.float32)
    bv_pair = np.concatenate([bv[h0], bv[h1]])          # [128]
    bvb8 = np.tile(bv_pair[None, :], (PD, 8)).astype(np.float32)
    wo2 = np.stack([Wo[h0 * Dh:(h0 + 1) * Dh], Wo[h1 * Dh:(h1 + 1) * Dh]])
    return {
        "xtb": xT_c,
        "wqkv": wqkv.astype(BF),
        "bqk": bqk,
        "bvb8": bvb8,
        "wo2": wo2.astype(BF),
    }


_CACHE = {}


def _get_nc():
    if "nc" not in _CACHE:
        _CACHE["nc"] = build_program()
    return _CACHE["nc"]


def kernel(x, Wq, Wk, Wv, bq, bk, bv, Wo, bo, _trace=False, _trace_kwargs=None):
    x, Wq, Wk, Wv, bq, bk, bv, Wo, bo = (
        np.asarray(a, np.float32) for a in (x, Wq, Wk, Wv, bq, bk, bv, Wo, bo)
    )
    nc = _get_nc()
    in_maps = [
        prep_core_inputs(c, x, Wq, Wk, Wv, bq, bk, bv, Wo) for c in range(NCORES)
    ]
    res = run_bass_kernel_spmd(
        nc, in_maps, list(range(NCORES)), trace=_trace, **(_trace_kwargs or {})
    )
    acc = np.asarray(res.results[0]["out"], np.float32)
    for c in range(1, NCORES):
        acc = acc + np.asarray(res.results[c]["out"], np.float32)
    acc += bo[None, None, :]
    if _trace:
        _CACHE["last_results"] = res
    return acc


# revision 4
# speedup vs baseline: 1.2138x; 1.2138x over previous
"""Head-parallel multi-head attention on 8 Trainium2 NeuronCores (v3).

Sharding: 2 heads per core (head axis split across 8 cores). Each core
computes its heads' Q/K/V projections (block-diagonal 128x128 weights,
both heads packed on the partition axis), full attention for its 2
heads, and a per-head partial W_o projection over its 128 head-dims.
The host sums the 8 partial outputs (the all-gather + W_o is
algebraically a sum of per-core partial matmuls) and adds b_o.

v3 over v2 (~229us): kt-lag pipeline instead of phase-lag.
  * PV(p, kt) runs 2 k-tiles behind exp(p, kt) inside the SAME phase
    (v2 lagged a full phase), and normalize/outproj/DMA for phase p
    flush during phase p+1 (v2: p+2).  The drain tail after the last
    exp shrinks from ~42us to a few us.
  * startup: x is DMA'd in 512-col chunks and Q/K are produced in
    512-col quarter matmuls, so the first scores matmul only waits for
    one small DMA chain; first exp fires ~3us in (v2: ~16us).
  * softmax 1/denom comes from a single DVE reciprocal (bf16 out)
    instead of the two-ACTIVATE exp(-ln d) chain: ACT runs only the
    128 [128,1024] exps (its hard floor, ~1.11us each).
  * per-phase PSUM stays at exactly 8 banks: scores ping-pong
    2x[128,1024] + oa(p)/oa(p-1) 2x[128,1024]; outproj(p-1) writes
    into retired oa(p-1) slices, late QKV parts reuse retired banks.
"""

import os
import sys
from contextlib import ExitStack

import numpy as np

for _p in ("/opt/trn_rl_repo", os.path.expanduser("~/.axon_site/_ro/trn_rl_repo")):
    if os.path.isdir(_p) and _p not in sys.path:
        sys.path.append(_p)

import ml_dtypes

import concourse.bass as bass
import concourse.tile as tile
from concourse import mybir
from concourse.bass_utils import run_bass_kernel_spmd

B, S, E, H = 2, 2048, 1024, 16
Dh = E // H           # 64
NCORES = 8
HPC = H // NCORES     # 2 heads per core
PD = HPC * Dh         # 128 pair dims per core
QC = 512              # q-chunk width
NQC = S // QC         # 4
KT = 128              # k-tile rows
NKT = S // KT         # 16
F32 = mybir.dt.float32
BF16 = mybir.dt.bfloat16
EXP = mybir.ActivationFunctionType.Exp
LN = mybir.ActivationFunctionType.Ln
BF = ml_dtypes.bfloat16

# 1/denom on DVE (one InstReciprocal) vs ACT exp(-ln d) fallback.
# Measured: InstReciprocal is ~8 cycles/element (6.55us per [1,1024]) and
# head-of-line-blocks the DVE queue, stalling ACT ~5us per phase. Keep False.
USE_DVE_RECIP = False


def split_multi_waits(nc):
    """Split multi-wait instructions into chained single-wait EventSemaphores.

    The walrus build here accepts at most ONE sync-wait command per
    instruction, while Tile emits several. Rewrite each instruction with
    N>1 waits into (N-1) same-engine EventSemaphore instructions (one
    wait each) followed by the instruction keeping its last wait --
    per-engine program order makes this equivalent.
    """
    n_split = 0
    for f in nc.m.functions:
        for blk in f.blocks:
            insts = list(blk.instructions)
            new = []
            for inst in insts:
                si = inst.sync_info
                waits = list(si.on_wait) if si is not None and si.on_wait else []
                if len(waits) > 1:
                    for j, w in enumerate(waits[:-1]):
                        ev = mybir.InstEventSemaphore(
                            name=f"{inst.name}-wsplit{j}", ins=[], outs=[]
                        )
                        ev.engine = inst.engine
                        ev.sync_info = mybir.SyncInfo(on_wait=[w], on_update=[])
                        nc.register_instruction(ev, overwrite=True)
                        new.append(ev)
                    si.on_wait = waits[-1:]
                    n_split += 1
                new.append(inst)
            blk.instructions = new
    return n_split


def build_program():
    nc = bass.Bass("TRN2", target_bir_lowering=False, debug=False)

    xtb = nc.dram_tensor("xtb", [B, PD, S], BF16, kind="ExternalInput").ap()
    wqkv = nc.dram_tensor("wqkv", [3, PD, PD], BF16, kind="ExternalInput").ap()
    bqk = nc.dram_tensor("bqk", [2, PD, 1], F32, kind="ExternalInput").ap()
    bvb8 = nc.dram_tensor("bvb8", [PD, 8 * PD], F32, kind="ExternalInput").ap()
    wo2 = nc.dram_tensor("wo2", [HPC, Dh, E], BF16, kind="ExternalInput").ap()
    out = nc.dram_tensor("out", [B, S, E], BF16, kind="ExternalOutput").ap()

    with tile.TileContext(nc) as tc, ExitStack() as ctx:
        const = ctx.enter_context(tc.tile_pool(name="const", bufs=1))
        perb = ctx.enter_context(tc.tile_pool(name="perb", bufs=2))
        slabp = ctx.enter_context(tc.tile_pool(name="slab", bufs=6))
        normp = ctx.enter_context(tc.tile_pool(name="norm", bufs=2))
        outp = ctx.enter_context(tc.tile_pool(name="outp", bufs=4))
        ps = ctx.enter_context(tc.tile_pool(name="ps", bufs=2, space="PSUM"))

        # ---- constants: order so the first Q/K quarter's deps land first ----
        w_sb = []
        for i in range(3):
            w_sb.append(const.tile([PD, PD], BF16, tag=f"w{i}", name=f"w{i}"))
        nc.gpsimd.dma_start(out=w_sb[0][:], in_=wqkv[0])
        nc.gpsimd.dma_start(out=w_sb[1][:], in_=wqkv[1])
        xtb_sb = const.tile([PD, B, S], BF16)
        nc.sync.dma_start(out=xtb_sb[:, 0, 0:QC], in_=xtb[0, :, 0:QC])
        bq_sb = const.tile([PD, 1], F32, tag="bq")
        nc.sync.dma_start(out=bq_sb[:], in_=bqk[0])
        bk_sb = const.tile([PD, 1], F32, tag="bk")
        nc.sync.dma_start(out=bk_sb[:], in_=bqk[1])
        nc.gpsimd.dma_start(out=w_sb[2][:], in_=wqkv[2])
        bvb8_sb = const.tile([PD, 8 * PD], F32, tag="bvb8")
        nc.gpsimd.dma_start(out=bvb8_sb[:], in_=bvb8)
        for g in range(1, 4):
            nc.sync.dma_start(out=xtb_sb[:, 0, g * QC:(g + 1) * QC],
                              in_=xtb[0, :, g * QC:(g + 1) * QC])
        nc.gpsimd.dma_start(out=xtb_sb[:, 1, :], in_=xtb[1])
        wop_sb = const.tile([PD, E], BF16, tag="wop")
        for h in range(HPC):
            nc.gpsimd.dma_start(out=wop_sb[h * Dh:(h + 1) * Dh, :], in_=wo2[h])
        ones_sb = const.tile([1, Dh], BF16, tag="ones")
        nc.vector.memset(ones_sb[:], 1.0)

        # ---- pipeline state ----
        phases = [(b, c) for b in range(B) for c in range(NQC)]
        NP = len(phases)
        slabs = {}   # (pi, kt) -> slab tile
        oas = {}     # phase idx -> oa PSUM tile [128, 1024] (rows 0:65 used)
        qts = {}     # batch -> qt tile
        kts = {}     # batch -> kt tile
        vaugs = {}   # batch -> vaug tile

        def emit_qkv_alloc(b):
            qt = perb.tile([PD, S], BF16, tag="qt", name=f"qt{b}")
            kt_t = perb.tile([PD, S], BF16, tag="kt", name=f"kt{b}")
            vaug = perb.tile([PD, NKT, HPC, Dh + 1], BF16, tag="vaug",
                             name=f"vaug{b}")
            nc.vector.memset(vaug[:, :, :, Dh], 1.0)
            qts[b], kts[b], vaugs[b] = qt, kt_t, vaug

        def qkv_quarter(b, kind, g, p=None):
            """One 512-wide Q (kind 0) or K (kind 1) quarter for batch b."""
            if p is None:
                p = ps.tile([PD, 2 * QC], F32, tag="scs", name="qkvps")
            dst = qts[b] if kind == 0 else kts[b]
            bias = bq_sb if kind == 0 else bk_sb
            sl_ = slice(g * QC, (g + 1) * QC)
            nc.tensor.matmul(p[:, 0:QC], lhsT=w_sb[kind][:],
                             rhs=xtb_sb[:, b, sl_])
            nc.vector.tensor_scalar_add(dst[:, sl_], p[:, 0:QC], bias[:])

        def qkv_half(b, kind, g, p=None):
            """One 1024-wide Q/K half (kind 0/1) or V half (kind 2)."""
            if p is None:
                p = ps.tile([PD, 2 * QC], F32, tag="scs", name="qkvps")
            if kind < 2:
                dst = qts[b] if kind == 0 else kts[b]
                bias = bq_sb if kind == 0 else bk_sb
                for j in range(2):
                    sl_ = slice((2 * g + j) * QC, (2 * g + j + 1) * QC)
                    nc.tensor.matmul(p[:, j * QC:(j + 1) * QC],
                                     lhsT=w_sb[kind][:],
                                     rhs=xtb_sb[:, b, sl_])
                nc.vector.tensor_scalar_add(
                    dst[:, 2 * g * QC:(2 * g + 2) * QC], p[:], bias[:])
            else:
                for i in range(8):
                    st = 8 * g + i
                    nc.tensor.matmul(p[:, i * PD:(i + 1) * PD],
                                     lhsT=xtb_sb[:, b, st * KT:(st + 1) * KT],
                                     rhs=w_sb[2][:])
                nc.vector.tensor_add(
                    vaugs[b][:, 8 * g:8 * (g + 1), :, 0:Dh],
                    p[:].rearrange("p (t h d) -> p t h d", t=8, h=HPC),
                    bvb8_sb[:].rearrange("p (t h d) -> p t h d", t=8, h=HPC),
                )

        def v_eighth(b, e, p=None):
            """V projection for k-tiles 4e..4e+3 (512 cols of x)."""
            if p is None:
                p = ps.tile([PD, 2 * QC], F32, tag="scs", name="qkvps")
            for i in range(4):
                st = 4 * e + i
                nc.tensor.matmul(p[:, i * PD:(i + 1) * PD],
                                 lhsT=xtb_sb[:, b, st * KT:(st + 1) * KT],
                                 rhs=w_sb[2][:])
            nc.vector.tensor_add(
                vaugs[b][:, 4 * e:4 * (e + 1), :, 0:Dh],
                p[:, 0:4 * PD].rearrange("p (t h d) -> p t h d", t=4, h=HPC),
                bvb8_sb[:, 0:4 * PD].rearrange("p (t h d) -> p t h d", t=4, h=HPC),
            )

        def emit_pv(pi, kt):
            oa = oas[pi]
            sl = slabs.pop((pi, kt))
            for h in range(HPC):
                nc.tensor.matmul(
                    oa[0:Dh + 1, h * QC:(h + 1) * QC],
                    lhsT=vaugs[phases[pi][0]][:, kt, h, :],
                    rhs=sl[:, h * QC:(h + 1) * QC],
                    start=(kt == 0), stop=(kt == NKT - 1),
                )

        def emit_recip(pi):
            """1/denom for phase pi: oa row 64 -> rr [1, 1024] bf16."""
            oa = oas[pi]
            rr = normp.tile([1, 2 * QC], BF16, tag="rr", name="rr")
            if USE_DVE_RECIP:
                with nc.allow_low_precision(reason="bf16 1/denom"):
                    nc.vector.reciprocal(rr[:], oa[Dh:Dh + 1, :])
            else:
                lnd = normp.tile([1, 2 * QC], F32, tag="lnd", name="lnd")
                nc.scalar.activation(lnd[:], oa[Dh:Dh + 1, :], LN)
                nc.scalar.activation(rr[:], lnd[:], EXP, scale=-1.0)
            return rr

        def emit_bcast(pi, rr):
            """Broadcast 1/denom across 64 partitions into oa rows 64:128."""
            oa = oas[pi]
            oa64 = oa[Dh:Dh + Dh, :]
            for h in range(HPC):
                nc.tensor.matmul(oa64[:, h * QC:(h + 1) * QC],
                                 lhsT=ones_sb[:],
                                 rhs=rr[:, h * QC:(h + 1) * QC],
                                 tile_position=(0, Dh))

        def emit_otp(pi):
            """Per-head normalized output: otp [128, 512] bf16 (h1 -> rows 64+)."""
            oa = oas[pi]
            oa64 = oa[Dh:Dh + Dh, :]
            bc = normp.tile([Dh, 2 * QC], F32, tag="bc", name="bc")
            nc.vector.tensor_copy(bc[:], oa64[:])
            otp = normp.tile([PD, QC], BF16, tag="otp", name="otp")
            for h in range(HPC):
                nc.vector.tensor_mul(otp[h * Dh:(h + 1) * Dh, :],
                                     oa[0:Dh, h * QC:(h + 1) * QC],
                                     bc[:, h * QC:(h + 1) * QC])
            return otp

        def emit_outproj_pair(pi, otp, i, outsb, slices=None):
            """Out-projection pair #i (stile i//2, echunk i%2) for phase pi."""
            oa = oas[pi]
            st, ec = i // 2, i % 2
            esl = slice(ec * QC, (ec + 1) * QC)
            if slices is None:
                sl_ = oa[:, ec * QC:(ec + 1) * QC]
            else:
                sl_ = slices[i % len(slices)]
            nc.tensor.matmul(sl_, lhsT=otp[:, st * KT:(st + 1) * KT],
                             rhs=wop_sb[:, esl])
            if slices is not None and i % 2 == 1:
                # tail only: ACT is idle after the last exp; split staging
                nc.scalar.copy(outsb[:, esl], sl_)
            else:
                nc.vector.tensor_copy(outsb[:, esl], sl_)

        def flush(fp, slot, state):
            """Flush work for phase fp, scheduled at kt position `slot`
            of the following phase (or compressed in the tail)."""
            if fp < 0:
                return
            if slot == 3:
                state["rr"] = emit_recip(fp)
            elif slot == 4:
                emit_bcast(fp, state["rr"])
            elif slot == 5:
                state["ots"] = emit_otp(fp)
            elif 6 <= slot < 14:
                i = slot - 6
                b2, c2 = phases[fp]
                st, ec = i // 2, i % 2
                if ec == 0:
                    state["outsb"] = outp.tile([KT, E], BF16, tag="outsb",
                                               name="outsb")
                emit_outproj_pair(fp, state["ots"], i, state["outsb"],
                                  slices=state.get("slices"))
                ssl = slice(c2 * QC + st * KT, c2 * QC + (st + 1) * KT)
                if state.get("slices") is not None:
                    # tail: DMA each echunk half as its copy lands
                    esl = slice(ec * QC, (ec + 1) * QC)
                    eng = (nc.sync, nc.gpsimd, nc.scalar)[i % 3]
                    eng.dma_start(out=out[b2, ssl, esl],
                                  in_=state["outsb"][:, esl])
                elif ec == 1:
                    eng = nc.sync if st % 2 == 0 else nc.gpsimd
                    eng.dma_start(out=out[b2, ssl, :], in_=state["outsb"][:])

        # QKV extras: (pi, kt) -> emit fn.  Phase 0 builds batch 0 piecewise;
        # batch 1 parts ride retired PSUM slots in phases (0,1)-(0,3).
        extras = {
            (0, 0): lambda: v_eighth(0, 0),
            (0, 1): lambda: qkv_quarter(0, 1, 1),
            (0, 2): lambda: v_eighth(0, 1),
            (0, 3): lambda: qkv_quarter(0, 1, 2),
            (0, 4): lambda: v_eighth(0, 2),
            (0, 5): lambda: qkv_quarter(0, 1, 3),
            (0, 6): lambda: v_eighth(0, 3),
            (0, 8): lambda: qkv_quarter(0, 0, 1),
            (0, 10): lambda: qkv_quarter(0, 0, 2),
            (0, 12): lambda: qkv_quarter(0, 0, 3),
            (1, 14): lambda: qkv_half(1, 1, 0),
            (1, 15): lambda: qkv_half(1, 0, 0),
            (2, 14): lambda: qkv_half(1, 1, 1, p=oas[1]),
            (2, 15): lambda: qkv_half(1, 0, 1, p=oas[1]),
            (3, 14): lambda: qkv_half(1, 2, 0, p=oas[2]),
            (3, 15): lambda: qkv_half(1, 2, 1, p=oas[2]),
        }

        state = {}
        emit_qkv_alloc(0)
        emit_qkv_alloc(1)
        qkv_quarter(0, 0, 0)   # Q chunk 0
        qkv_quarter(0, 1, 0)   # K k-tiles 0-3
        for pi, (b, c) in enumerate(phases):
            qt, kt_t = qts[b], kts[b]
            csl = slice(c * QC, (c + 1) * QC)
            oas[pi] = ps.tile([PD, 2 * QC], F32, tag="oa", name=f"oa{pi}")
            for kt in range(NKT):
                scs = ps.tile([PD, 2 * QC], F32, tag="scs", name="scs")
                for h in range(HPC):
                    hsl = slice(Dh * h, Dh * (h + 1))
                    # 2x row tiling: both heads stream concurrently
                    nc.tensor.matmul(
                        scs[:, h * QC:(h + 1) * QC],
                        lhsT=kt_t[hsl, kt * KT:(kt + 1) * KT],
                        rhs=qt[hsl, csl],
                        tile_position=(Dh * h, 0),
                    )
                sl_t = slabp.tile([PD, 2 * QC], BF16, tag="slab", name="slab")
                nc.scalar.activation(sl_t[:], scs[:], EXP, scale=0.125)
                slabs[(pi, kt)] = sl_t
                # PV: 2 k-tiles behind exp; first two slots finish pi-1
                if kt >= 2:
                    emit_pv(pi, kt - 2)
                elif pi >= 1:
                    emit_pv(pi - 1, NKT - 2 + kt)
                if pi >= 1:
                    flush(pi - 1, kt, state)
                ex = extras.get((pi, kt))
                if ex is not None:
                    ex()

        # ---- tail: finish PV for the last phase, flush it compressed ----
        last = NP - 1
        emit_pv(last, NKT - 2)
        emit_pv(last, NKT - 1)
        state["rr"] = emit_recip(last)
        emit_bcast(last, state["rr"])
        state["ots"] = emit_otp(last)
        # rotate through FOUR retired PSUM slices (oa last + last-1) so the
        # matmul->copy->DMA chain pipelines 4 deep
        state["slices"] = [
            oas[last][:, 0:QC], oas[last][:, QC:2 * QC],
            oas[last - 1][:, 0:QC], oas[last - 1][:, QC:2 * QC],
        ]
        for slot in range(6, 14):
            flush(last, slot, state)

    from concourse.library_overlay import lower_extended_insts

    lower_extended_insts(nc)
    split_multi_waits(nc)
    return nc


def prep_core_inputs(c, x, Wq, Wk, Wv, bq, bk, bv, Wo):
    h0, h1 = HPC * c, HPC * c + 1
    xT_c = np.ascontiguousarray(
        np.transpose(x[:, :, c * PD:(c + 1) * PD], (0, 2, 1))
    ).astype(BF)
    wqkv = np.zeros((3, PD, PD), np.float32)
    for i, W in enumerate((Wq, Wk, Wv)):
        wqkv[i, :Dh, :Dh] = W[h0]
        wqkv[i, Dh:, Dh:] = W[h1]
    bqk = np.stack([
        np.concatenate([bq[h0], bq[h1]])[:, None],
        np.concatenate([bk[h0], bk[h1]])[:, None],
    ]).astype(np.float32)
    bv_pair = np.concatenate([bv[h0], bv[h1]])          # [128]
    bvb8 = np.tile(bv_pair[None, :], (PD, 8)).astype(np.float32)
    wo2 = np.stack([Wo[h0 * Dh:(h0 + 1) * Dh], Wo[h1 * Dh:(h1 + 1) * Dh]])
    return {
        "xtb": xT_c,
        "wqkv": wqkv.astype(BF),
        "bqk": bqk,
        "bvb8": bvb8,
        "wo2": wo2.astype(BF),
    }


_CACHE = {}


def _get_nc():
    if "nc" not in _CACHE:
        _CACHE["nc"] = build_program()
    return _CACHE["nc"]


def kernel(x, Wq, Wk, Wv, bq, bk, bv, Wo, bo, _trace=False, _trace_kwargs=None):
    x, Wq, Wk, Wv, bq, bk, bv, Wo, bo = (
        np.asarray(a, np.float32) for a in (x, Wq, Wk, Wv, bq, bk, bv, Wo, bo)
    )
    nc = _get_nc()
    in_maps = [
        prep_core_inputs(c, x, Wq, Wk, Wv, bq, bk, bv, Wo) for c in range(NCORES)
    ]
    res = run_bass_kernel_spmd(
        nc, in_maps, list(range(NCORES)), trace=_trace, **(_trace_kwargs or {})
    )
    acc = np.asarray(res.results[0]["out"], np.float32)
    for c in range(1, NCORES):
        acc = acc + np.asarray(res.results[c]["out"], np.float32)
    acc += bo[None, None, :]
    if _trace:
        _CACHE["last_results"] = res
    return acc


# revision 14
# speedup vs baseline: 1.2458x; 1.0263x over previous
"""Head-parallel multi-head attention on 8 Trainium2 NeuronCores (v3).

Sharding: 2 heads per core (head axis split across 8 cores). Each core
computes its heads' Q/K/V projections (block-diagonal 128x128 weights,
both heads packed on the partition axis), full attention for its 2
heads, and a per-head partial W_o projection over its 128 head-dims.
The host sums the 8 partial outputs (the all-gather + W_o is
algebraically a sum of per-core partial matmuls) and adds b_o.

v3 over v2 (~229us): kt-lag pipeline instead of phase-lag.
  * PV(p, kt) runs 2 k-tiles behind exp(p, kt) inside the SAME phase
    (v2 lagged a full phase), and normalize/outproj/DMA for phase p
    flush during phase p+1 (v2: p+2).  The drain tail after the last
    exp shrinks from ~42us to a few us.
  * startup: x is DMA'd in 512-col chunks and Q/K are produced in
    512-col quarter matmuls, so the first scores matmul only waits for
    one small DMA chain; first exp fires ~3us in (v2: ~16us).
  * softmax 1/denom comes from a single DVE reciprocal (bf16 out)
    instead of the two-ACTIVATE exp(-ln d) chain: ACT runs only the
    128 [128,1024] exps (its hard floor, ~1.11us each).
  * per-phase PSUM stays at exactly 8 banks: scores ping-pong
    2x[128,1024] + oa(p)/oa(p-1) 2x[128,1024]; outproj(p-1) writes
    into retired oa(p-1) slices, late QKV parts reuse retired banks.
"""

import os
import sys
from contextlib import ExitStack

import numpy as np

for _p in ("/opt/trn_rl_repo", os.path.expanduser("~/.axon_site/_ro/trn_rl_repo")):
    if os.path.isdir(_p) and _p not in sys.path:
        sys.path.append(_p)

import ml_dtypes

import concourse.bass as bass
import concourse.tile as tile
from concourse import mybir
from concourse.bass_utils import run_bass_kernel_spmd

B, S, E, H = 2, 2048, 1024, 16
Dh = E // H           # 64
NCORES = 8
HPC = H // NCORES     # 2 heads per core
PD = HPC * Dh         # 128 pair dims per core
QC = 512              # q-chunk width
NQC = S // QC         # 4
KT = 128              # k-tile rows
VW = 96               # vaug width: v dims 0:64, zeros, denom-ones col at 95
NKT = S // KT         # 16
F32 = mybir.dt.float32
BF16 = mybir.dt.bfloat16
EXP = mybir.ActivationFunctionType.Exp
LN = mybir.ActivationFunctionType.Ln
BF = ml_dtypes.bfloat16

# 1/denom strategies.  Measured: a [1,1024] InstReciprocal is ~8 cyc/elem
# (6.55us) and head-of-line-blocks the DVE queue -> never use it wide.
# TRANS: DVE 32x32 stream-transpose the denom row into a [32,32]-strided
# layout, reciprocal at free-size 32 (~0.2us), transpose back.  Fallback:
# ACT exp(-ln d) chain (2.2us of ACT per phase).
RECIP_MODE = "trans"  # "trans" | "act"


def split_multi_waits(nc):
    """Split multi-wait instructions into chained single-wait EventSemaphores.

    The walrus build here accepts at most ONE sync-wait command per
    instruction, while Tile emits several. Rewrite each instruction with
    N>1 waits into (N-1) same-engine EventSemaphore instructions (one
    wait each) followed by the instruction keeping its last wait --
    per-engine program order makes this equivalent.
    """
    n_split = 0
    for f in nc.m.functions:
        for blk in f.blocks:
            insts = list(blk.instructions)
            new = []
            for inst in insts:
                si = inst.sync_info
                waits = list(si.on_wait) if si is not None and si.on_wait else []
                if len(waits) > 1:
                    for j, w in enumerate(waits[:-1]):
                        ev = mybir.InstEventSemaphore(
                            name=f"{inst.name}-wsplit{j}", ins=[], outs=[]
                        )
                        ev.engine = inst.engine
                        ev.sync_info = mybir.SyncInfo(on_wait=[w], on_update=[])
                        nc.register_instruction(ev, overwrite=True)
                        new.append(ev)
                    si.on_wait = waits[-1:]
                    n_split += 1
                new.append(inst)
            blk.instructions = new
    return n_split


def build_program():
    nc = bass.Bass("TRN2", target_bir_lowering=False, debug=False)

    xtb = nc.dram_tensor("xtb", [B, PD, S], BF16, kind="ExternalInput").ap()
    wqkv = nc.dram_tensor("wqkv", [3, PD, PD], BF16, kind="ExternalInput").ap()
    bqk = nc.dram_tensor("bqk", [2, PD, 1], F32, kind="ExternalInput").ap()
    bvb8 = nc.dram_tensor("bvb8", [PD, 8 * PD], F32, kind="ExternalInput").ap()
    wo2 = nc.dram_tensor("wo2", [HPC, Dh, E], BF16, kind="ExternalInput").ap()
    out = nc.dram_tensor("out", [B, S, E], BF16, kind="ExternalOutput").ap()

    with tile.TileContext(nc) as tc, ExitStack() as ctx:
        const = ctx.enter_context(tc.tile_pool(name="const", bufs=1))
        perb = ctx.enter_context(tc.tile_pool(name="perb", bufs=2))
        slabp = ctx.enter_context(tc.tile_pool(name="slab", bufs=6))
        normp = ctx.enter_context(tc.tile_pool(name="norm", bufs=2))
        outp = ctx.enter_context(tc.tile_pool(name="outp", bufs=4))
        ps = ctx.enter_context(tc.tile_pool(name="ps", bufs=2, space="PSUM"))

        # ---- constants: order so the first Q/K quarter's deps land first ----
        w_sb = []
        for i in range(3):
            w_sb.append(const.tile([PD, PD], BF16, tag=f"w{i}", name=f"w{i}"))
        nc.gpsimd.dma_start(out=w_sb[0][:], in_=wqkv[0])
        nc.gpsimd.dma_start(out=w_sb[1][:], in_=wqkv[1])
        xtb_sb = const.tile([PD, B, S], BF16)
        nc.sync.dma_start(out=xtb_sb[:, 0, 0:QC], in_=xtb[0, :, 0:QC])
        bq_sb = const.tile([PD, 1], F32, tag="bq")
        nc.sync.dma_start(out=bq_sb[:], in_=bqk[0])
        bk_sb = const.tile([PD, 1], F32, tag="bk")
        nc.sync.dma_start(out=bk_sb[:], in_=bqk[1])
        nc.gpsimd.dma_start(out=w_sb[2][:], in_=wqkv[2])
        bvb8_sb = const.tile([PD, 8 * PD], F32, tag="bvb8")
        nc.gpsimd.dma_start(out=bvb8_sb[:], in_=bvb8)
        for g in range(1, 4):
            nc.sync.dma_start(out=xtb_sb[:, 0, g * QC:(g + 1) * QC],
                              in_=xtb[0, :, g * QC:(g + 1) * QC])
        nc.gpsimd.dma_start(out=xtb_sb[:, 1, :], in_=xtb[1])
        wop_sb = const.tile([PD, E], BF16, tag="wop")
        for h in range(HPC):
            nc.gpsimd.dma_start(out=wop_sb[h * Dh:(h + 1) * Dh, :], in_=wo2[h])
        ones_sb = const.tile([1, Dh], BF16, tag="ones")
        nc.vector.memset(ones_sb[:], 1.0)
        rrT_sb = const.tile([32, 2 * QC], BF16, tag="rrT")
        nc.vector.memset(rrT_sb[:], 0.0)

        # ---- pipeline state ----
        phases = [(b, c) for b in range(B) for c in range(NQC)]
        NP = len(phases)
        slabs = {}   # (pi, kt) -> slab tile
        oas = {}     # phase idx -> oa PSUM tile [128, 1024] (rows 0:65 used)
        qts = {}     # batch -> qt tile
        kts = {}     # batch -> kt tile
        vaugs = {}   # batch -> vaug tile

        def emit_qkv_alloc(b):
            qt = perb.tile([PD, S], BF16, tag="qt", name=f"qt{b}")
            kt_t = perb.tile([PD, S], BF16, tag="kt", name=f"kt{b}")
            vaug = perb.tile([PD, NKT, HPC, VW], BF16, tag="vaug",
                             name=f"vaug{b}")
            nc.vector.memset(vaug[:, :, :, Dh:VW - 1], 0.0)
            nc.vector.memset(vaug[:, :, :, VW - 1], 1.0)
            qts[b], kts[b], vaugs[b] = qt, kt_t, vaug

        # QKV projection pieces.  Mid-phase pieces ride the scs PSUM ring,
        # which the scores/exp ping-pong also uses; every piece consumes an
        # EVEN number of ring slots (real + dummy, or two real) so scores
        # parity is preserved -- an odd-slot piece makes the next scores
        # matmul WAR-wait on a 1-tile-old exp (~0.7us ACT stall each).
        def qkv_quarter(b, kind, g, p=None, pad=True):
            """One 512-wide Q (kind 0) or K (kind 1) quarter for batch b."""
            if p is None:
                p = ps.tile([PD, 2 * QC], F32, tag="scs", name="qkvps")
                if pad:
                    ps.tile([PD, 2 * QC], F32, tag="scs", name="par")
            dst = qts[b] if kind == 0 else kts[b]
            bias = bq_sb if kind == 0 else bk_sb
            sl_ = slice(g * QC, (g + 1) * QC)
            nc.tensor.matmul(p[:, 0:QC], lhsT=w_sb[kind][:],
                             rhs=xtb_sb[:, b, sl_])
            nc.vector.tensor_scalar_add(dst[:, sl_], p[:, 0:QC], bias[:])

        def qkv_half(b, kind, g, p=None):
            """One 1024-wide Q/K half (kind 0/1) or V half (kind 2),
            split over two ring slots (parity-neutral)."""
            if kind < 2:
                dst = qts[b] if kind == 0 else kts[b]
                bias = bq_sb if kind == 0 else bk_sb
                for j in range(2):
                    pj = p if p is not None else ps.tile(
                        [PD, 2 * QC], F32, tag="scs", name="qkvps")
                    sl_ = slice((2 * g + j) * QC, (2 * g + j + 1) * QC)
                    nc.tensor.matmul(pj[:, j * QC:(j + 1) * QC],
                                     lhsT=w_sb[kind][:],
                                     rhs=xtb_sb[:, b, sl_])
                    nc.vector.tensor_scalar_add(
                        dst[:, sl_], pj[:, j * QC:(j + 1) * QC], bias[:])
            else:
                for j in range(2):
                    pj = p if p is not None else ps.tile(
                        [PD, 2 * QC], F32, tag="scs", name="qkvps")
                    v_quad(b, 2 * g + j, pj, j)

        def v_quad(b, q, p, half):
            """V projection for k-tiles 4q..4q+3 into half `half` of p."""
            o = half * 4 * PD
            for i in range(4):
                st = 4 * q + i
                nc.tensor.matmul(p[:, o + i * PD:o + (i + 1) * PD],
                                 lhsT=xtb_sb[:, b, st * KT:(st + 1) * KT],
                                 rhs=w_sb[2][:])
            nc.vector.tensor_add(
                vaugs[b][:, 4 * q:4 * (q + 1), :, 0:Dh],
                p[:, o:o + 4 * PD].rearrange("p (t h d) -> p t h d", t=4, h=HPC),
                bvb8_sb[:, 0:4 * PD].rearrange("p (t h d) -> p t h d", t=4, h=HPC),
            )

        def v_eighth(b, e):
            """V projection for k-tiles 4e..4e+3, split over two ring slots
            (2 matmuls + drain each) to stay parity-neutral."""
            for half in range(2):
                p = ps.tile([PD, 2 * QC], F32, tag="scs", name="qkvps")
                o = half * 2
                for i in range(2):
                    st = 4 * e + o + i
                    nc.tensor.matmul(p[:, i * PD:(i + 1) * PD],
                                     lhsT=xtb_sb[:, b, st * KT:(st + 1) * KT],
                                     rhs=w_sb[2][:])
                nc.vector.tensor_add(
                    vaugs[b][:, 4 * e + o:4 * e + o + 2, :, 0:Dh],
                    p[:, 0:2 * PD].rearrange("p (t h d) -> p t h d", t=2, h=HPC),
                    bvb8_sb[:, 0:2 * PD].rearrange("p (t h d) -> p t h d",
                                                   t=2, h=HPC),
                )

        def emit_pv(pi, kt):
            oa = oas[pi]
            sl = slabs.pop((pi, kt))
            for h in range(HPC):
                nc.tensor.matmul(
                    oa[0:VW, h * QC:(h + 1) * QC],
                    lhsT=vaugs[phases[pi][0]][:, kt, h, :],
                    rhs=sl[:, h * QC:(h + 1) * QC],
                    start=(kt == 0), stop=(kt == NKT - 1),
                )

        def emit_recip(pi):
            """1/denom for phase pi: oa row 64 -> rr [1 (or 32), 1024] bf16."""
            oa = oas[pi]
            if RECIP_MODE == "trans":
                # 32x32 block transpose of oa rows 33:65 (all PV-written)
                # scatters the denom row (block-row 31) across partitions:
                # dnmT[i, 32b+31] = denom[32b+i].  Reciprocal runs on that
                # stride-32 view (free-size 32, ~0.2us); transposing back
                # packs row 31 = 1/denom.  rrT_sb is memset once and only
                # its 31-columns are rewritten each phase, keeping the
                # transpose-back input fully initialized.
                dnmT = normp.tile([32, 2 * QC], F32, tag="dnmT", name="dnmT")
                nc.vector.transpose(dnmT[:], oa[Dh:Dh + 32, :])
                # write 1/denom into c-position 0 of each block so the
                # transpose back packs it into row 0 (matmul rhs needs a
                # 32-aligned start partition)
                with nc.allow_low_precision(reason="bf16 1/denom"):
                    nc.vector.reciprocal(
                        rrT_sb[:].rearrange("p (b c) -> p b c", c=32)[:, :, 0],
                        dnmT[:].rearrange("p (b c) -> p b c", c=32)[:, :, 31],
                    )
                rr = normp.tile([32, 2 * QC], BF16, tag="rr", name="rr")
                nc.vector.transpose(rr[:], rrT_sb[:])
                return rr, 0
            rr = normp.tile([1, 2 * QC], BF16, tag="rr", name="rr")
            lnd = normp.tile([1, 2 * QC], F32, tag="lnd", name="lnd")
            nc.scalar.activation(lnd[:], oa[VW - 1:VW, :], LN)
            nc.scalar.activation(rr[:], lnd[:], EXP, scale=-1.0)
            return rr, 0

        def emit_bcast(pi, rrow):
            """Broadcast 1/denom across 64 partitions into oa rows 64:128."""
            oa = oas[pi]
            oa64 = oa[Dh:Dh + Dh, :]
            rr, row = rrow
            for h in range(HPC):
                nc.tensor.matmul(oa64[:, h * QC:(h + 1) * QC],
                                 lhsT=ones_sb[:],
                                 rhs=rr[row:row + 1, h * QC:(h + 1) * QC],
                                 tile_position=(0, Dh))

        def emit_otp(pi):
            """Per-head normalized output: otp [128, 512] bf16 (h1 -> rows 64+)."""
            oa = oas[pi]
            oa64 = oa[Dh:Dh + Dh, :]
            bc = normp.tile([Dh, 2 * QC], F32, tag="bc", name="bc")
            nc.vector.tensor_copy(bc[:], oa64[:])
            otp = normp.tile([PD, QC], BF16, tag="otp", name="otp")
            for h in range(HPC):
                nc.vector.tensor_mul(otp[h * Dh:(h + 1) * Dh, :],
                                     oa[0:Dh, h * QC:(h + 1) * QC],
                                     bc[:, h * QC:(h + 1) * QC])
            return otp

        def emit_outproj_pair(pi, otp, i, outsb, slices=None):
            """Out-projection pair #i (stile i//2, echunk i%2) for phase pi."""
            oa = oas[pi]
            st, ec = i // 2, i % 2
            esl = slice(ec * QC, (ec + 1) * QC)
            if slices is None:
                sl_ = oa[:, ec * QC:(ec + 1) * QC]
            else:
                sl_ = slices[i % len(slices)]
            nc.tensor.matmul(sl_, lhsT=otp[:, st * KT:(st + 1) * KT],
                             rhs=wop_sb[:, esl])
            if slices is not None and i % 2 == 1:
                # tail only: ACT is idle after the last exp; split staging
                nc.scalar.copy(outsb[:, esl], sl_)
            else:
                nc.vector.tensor_copy(outsb[:, esl], sl_)

        def flush(fp, slot, state):
            """Flush work for phase fp, scheduled at kt position `slot`
            of the following phase (or compressed in the tail)."""
            if fp < 0:
                return
            if slot == 3:
                state["rr"] = emit_recip(fp)
            elif slot == 4:
                emit_bcast(fp, state["rr"])
            elif slot == 5:
                state["ots"] = emit_otp(fp)
            elif 6 <= slot < 14:
                i = slot - 6
                b2, c2 = phases[fp]
                st, ec = i // 2, i % 2
                if ec == 0:
                    state["outsb"] = outp.tile([KT, E], BF16, tag="outsb",
                                               name="outsb")
                emit_outproj_pair(fp, state["ots"], i, state["outsb"],
                                  slices=state.get("slices"))
                ssl = slice(c2 * QC + st * KT, c2 * QC + (st + 1) * KT)
                if state.get("slices") is not None:
                    # tail: DMA each echunk half as its copy lands
                    esl = slice(ec * QC, (ec + 1) * QC)
                    eng = (nc.sync, nc.gpsimd, nc.scalar)[i % 3]
                    eng.dma_start(out=out[b2, ssl, esl],
                                  in_=state["outsb"][:, esl])
                elif ec == 1:
                    eng = nc.sync if st % 2 == 0 else nc.gpsimd
                    eng.dma_start(out=out[b2, ssl, :], in_=state["outsb"][:])

        # QKV extras: (pi, kt) -> emit fn.  Phase 0 builds batch 0 piecewise;
        # batch 1 parts ride retired PSUM slots in phases (0,1)-(0,3).
        extras = {
            (0, 0): lambda: v_eighth(0, 0),
            (0, 1): lambda: qkv_quarter(0, 1, 1),
            (0, 2): lambda: v_eighth(0, 1),
            (0, 3): lambda: qkv_quarter(0, 1, 2),
            (0, 4): lambda: v_eighth(0, 2),
            (0, 5): lambda: qkv_quarter(0, 1, 3),
            (0, 6): lambda: v_eighth(0, 3),
            (0, 8): lambda: qkv_quarter(0, 0, 1),
            (0, 10): lambda: qkv_quarter(0, 0, 2),
            (0, 12): lambda: qkv_quarter(0, 0, 3),
            (1, 14): lambda: qkv_half(1, 1, 0),
            (1, 15): lambda: qkv_half(1, 0, 0),
            (2, 14): lambda: qkv_half(1, 1, 1, p=oas[1]),
            (2, 15): lambda: qkv_half(1, 0, 1, p=oas[1]),
            (3, 14): lambda: qkv_half(1, 2, 0, p=oas[2]),
            (3, 15): lambda: qkv_half(1, 2, 1, p=oas[2]),
        }

        state = {}
        emit_qkv_alloc(0)
        emit_qkv_alloc(1)
        qkv_quarter(0, 0, 0, pad=False)   # Q chunk 0
        qkv_quarter(0, 1, 0, pad=False)   # K k-tiles 0-3
        for pi, (b, c) in enumerate(phases):
            qt, kt_t = qts[b], kts[b]
            csl = slice(c * QC, (c + 1) * QC)
            oas[pi] = ps.tile([PD, 2 * QC], F32, tag="oa", name=f"oa{pi}")
            for kt in range(NKT):
                scs = ps.tile([PD, 2 * QC], F32, tag="scs", name="scs")
                for h in range(HPC):
                    hsl = slice(Dh * h, Dh * (h + 1))
                    # 2x row tiling: both heads stream concurrently
                    nc.tensor.matmul(
                        scs[:, h * QC:(h + 1) * QC],
                        lhsT=kt_t[hsl, kt * KT:(kt + 1) * KT],
                        rhs=qt[hsl, csl],
                        tile_position=(Dh * h, 0),
                    )
                sl_t = slabp.tile([PD, 2 * QC], BF16, tag="slab", name="slab")
                nc.scalar.activation(sl_t[:], scs[:], EXP, scale=0.125)
                slabs[(pi, kt)] = sl_t
                # PV: 2 k-tiles behind exp; first two slots finish pi-1
                if kt >= 2:
                    emit_pv(pi, kt - 2)
                elif pi >= 1:
                    emit_pv(pi - 1, NKT - 2 + kt)
                if pi >= 1:
                    flush(pi - 1, kt, state)
                ex = extras.get((pi, kt))
                if ex is not None:
                    ex()

        # ---- tail: finish PV for the last phase, flush it compressed ----
        last = NP - 1
        emit_pv(last, NKT - 2)
        emit_pv(last, NKT - 1)
        state["rr"] = emit_recip(last)
        emit_bcast(last, state["rr"])
        state["ots"] = emit_otp(last)
        # rotate through FOUR retired PSUM slices (oa last + last-1) so the
        # matmul->copy->DMA chain pipelines 4 deep
        state["slices"] = [
            oas[last][:, 0:QC], oas[last][:, QC:2 * QC],
            oas[last - 1][:, 0:QC], oas[last - 1][:, QC:2 * QC],
        ]
        for slot in range(6, 14):
            flush(last, slot, state)

    from concourse.library_overlay import lower_extended_insts

    lower_extended_insts(nc)
    split_multi_waits(nc)
    return nc


def prep_core_inputs(c, x, Wq, Wk, Wv, bq, bk, bv, Wo):
    h0, h1 = HPC * c, HPC * c + 1
    xT_c = np.ascontiguousarray(
        np.transpose(x[:, :, c * PD:(c + 1) * PD], (0, 2, 1))
    ).astype(BF)
    wqkv = np.zeros((3, PD, PD), np.float32)
    for i, W in enumerate((Wq, Wk, Wv)):
        wqkv[i, :Dh, :Dh] = W[h0]
        wqkv[i, Dh:, Dh:] = W[h1]
    bqk = np.stack([
        np.concatenate([bq[h0], bq[h1]])[:, None],
        np.concatenate([bk[h0], bk[h1]])[:, None],
    ]).astype(np.float32)
    bv_pair = np.concatenate([bv[h0], bv[h1]])          # [128]
    bvb8 = np.tile(bv_pair[None, :], (PD, 8)).astype(np.float32)
    wo2 = np.stack([Wo[h0 * Dh:(h0 + 1) * Dh], Wo[h1 * Dh:(h1 + 1) * Dh]])
    return {
        "xtb": xT_c,
        "wqkv": wqkv.astype(BF),
        "bqk": bqk,
        "bvb8": bvb8,
        "wo2": wo2.astype(BF),
    }


_CACHE = {}


def _get_nc():
    if "nc" not in _CACHE:
        _CACHE["nc"] = build_program()
    return _CACHE["nc"]


def kernel(x, Wq, Wk, Wv, bq, bk, bv, Wo, bo, _trace=False, _trace_kwargs=None):
    x, Wq, Wk, Wv, bq, bk, bv, Wo, bo = (
        np.asarray(a, np.float32) for a in (x, Wq, Wk, Wv, bq, bk, bv, Wo, bo)
    )
    nc = _get_nc()
    in_maps = [
        prep_core_inputs(c, x, Wq, Wk, Wv, bq, bk, bv, Wo) for c in range(NCORES)
    ]
    res = run_bass_kernel_spmd(
        nc, in_maps, list(range(NCORES)), trace=_trace, **(_trace_kwargs or {})
    )
    acc = np.asarray(res.results[0]["out"], np.float32)
    for c in range(1, NCORES):
        acc = acc + np.asarray(res.results[c]["out"], np.float32)
    acc += bo[None, None, :]
    if _trace:
        _CACHE["last_results"] = res
    return acc


# revision 20
# speedup vs baseline: 1.2525x; 1.0054x over previous
"""Head-parallel multi-head attention on 8 Trainium2 NeuronCores (v3).

Sharding: 2 heads per core (head axis split across 8 cores). Each core
computes its heads' Q/K/V projections (block-diagonal 128x128 weights,
both heads packed on the partition axis), full attention for its 2
heads, and a per-head partial W_o projection over its 128 head-dims.
The host sums the 8 partial outputs (the all-gather + W_o is
algebraically a sum of per-core partial matmuls) and adds b_o.

v3 over v2 (~229us): kt-lag pipeline instead of phase-lag.
  * PV(p, kt) runs 2 k-tiles behind exp(p, kt) inside the SAME phase
    (v2 lagged a full phase), and normalize/outproj/DMA for phase p
    flush during phase p+1 (v2: p+2).  The drain tail after the last
    exp shrinks from ~42us to a few us.
  * startup: x is DMA'd in 512-col chunks and Q/K are produced in
    512-col quarter matmuls, so the first scores matmul only waits for
    one small DMA chain; first exp fires ~3us in (v2: ~16us).
  * softmax 1/denom comes from a single DVE reciprocal (bf16 out)
    instead of the two-ACTIVATE exp(-ln d) chain: ACT runs only the
    128 [128,1024] exps (its hard floor, ~1.11us each).
  * per-phase PSUM stays at exactly 8 banks: scores ping-pong
    2x[128,1024] + oa(p)/oa(p-1) 2x[128,1024]; outproj(p-1) writes
    into retired oa(p-1) slices, late QKV parts reuse retired banks.
"""

import os
import sys
from contextlib import ExitStack

import numpy as np

for _p in ("/opt/trn_rl_repo", os.path.expanduser("~/.axon_site/_ro/trn_rl_repo")):
    if os.path.isdir(_p) and _p not in sys.path:
        sys.path.append(_p)

import ml_dtypes

import concourse.bass as bass
import concourse.tile as tile
from concourse import mybir
from concourse.bass_utils import run_bass_kernel_spmd

B, S, E, H = 2, 2048, 1024, 16
Dh = E // H           # 64
NCORES = 8
HPC = H // NCORES     # 2 heads per core
PD = HPC * Dh         # 128 pair dims per core
QC = 512              # q-chunk width
NQC = S // QC         # 4
KT = 128              # k-tile rows
VW = 96               # vaug width: v dims 0:64, zeros, denom-ones col at 95
NKT = S // KT         # 16
F32 = mybir.dt.float32
BF16 = mybir.dt.bfloat16
EXP = mybir.ActivationFunctionType.Exp
LN = mybir.ActivationFunctionType.Ln
BF = ml_dtypes.bfloat16

# 1/denom strategies.  Measured: a [1,1024] InstReciprocal is ~8 cyc/elem
# (6.55us) and head-of-line-blocks the DVE queue -> never use it wide.
# TRANS: DVE 32x32 stream-transpose the denom row into a [32,32]-strided
# layout, reciprocal at free-size 32 (~0.2us), transpose back.  Fallback:
# ACT exp(-ln d) chain (2.2us of ACT per phase).
RECIP_MODE = "trans"  # "trans" | "act"


def split_multi_waits(nc):
    """Split multi-wait instructions into chained single-wait EventSemaphores.

    The walrus build here accepts at most ONE sync-wait command per
    instruction, while Tile emits several. Rewrite each instruction with
    N>1 waits into (N-1) same-engine EventSemaphore instructions (one
    wait each) followed by the instruction keeping its last wait --
    per-engine program order makes this equivalent.
    """
    n_split = 0
    for f in nc.m.functions:
        for blk in f.blocks:
            insts = list(blk.instructions)
            new = []
            for inst in insts:
                si = inst.sync_info
                waits = list(si.on_wait) if si is not None and si.on_wait else []
                if len(waits) > 1:
                    for j, w in enumerate(waits[:-1]):
                        ev = mybir.InstEventSemaphore(
                            name=f"{inst.name}-wsplit{j}", ins=[], outs=[]
                        )
                        ev.engine = inst.engine
                        ev.sync_info = mybir.SyncInfo(on_wait=[w], on_update=[])
                        nc.register_instruction(ev, overwrite=True)
                        new.append(ev)
                    si.on_wait = waits[-1:]
                    n_split += 1
                new.append(inst)
            blk.instructions = new
    return n_split


def build_program():
    nc = bass.Bass("TRN2", target_bir_lowering=False, debug=False)

    xtb = nc.dram_tensor("xtb", [B, PD, S], BF16, kind="ExternalInput").ap()
    wqkv = nc.dram_tensor("wqkv", [3, PD, PD], BF16, kind="ExternalInput").ap()
    bqk = nc.dram_tensor("bqk", [2, PD, 1], F32, kind="ExternalInput").ap()
    bvb8 = nc.dram_tensor("bvb8", [PD, 8 * PD], F32, kind="ExternalInput").ap()
    wo2 = nc.dram_tensor("wo2", [HPC, Dh, E], BF16, kind="ExternalInput").ap()
    out = nc.dram_tensor("out", [B, S, E], BF16, kind="ExternalOutput").ap()

    with tile.TileContext(nc) as tc, ExitStack() as ctx:
        const = ctx.enter_context(tc.tile_pool(name="const", bufs=1))
        perb = ctx.enter_context(tc.tile_pool(name="perb", bufs=2))
        slabp = ctx.enter_context(tc.tile_pool(name="slab", bufs=6))
        normp = ctx.enter_context(tc.tile_pool(name="norm", bufs=2))
        outp = ctx.enter_context(tc.tile_pool(name="outp", bufs=4))
        ps = ctx.enter_context(tc.tile_pool(name="ps", bufs=2, space="PSUM"))

        # ---- constants: order so the first Q/K quarter's deps land first ----
        w_sb = []
        for i in range(3):
            w_sb.append(const.tile([PD, PD], BF16, tag=f"w{i}", name=f"w{i}"))
        nc.gpsimd.dma_start(out=w_sb[0][:], in_=wqkv[0])
        nc.gpsimd.dma_start(out=w_sb[1][:], in_=wqkv[1])
        xtb_sb = const.tile([PD, B, S], BF16)
        nc.sync.dma_start(out=xtb_sb[:, 0, 0:QC], in_=xtb[0, :, 0:QC])
        bq_sb = const.tile([PD, 1], F32, tag="bq")
        nc.sync.dma_start(out=bq_sb[:], in_=bqk[0])
        bk_sb = const.tile([PD, 1], F32, tag="bk")
        nc.sync.dma_start(out=bk_sb[:], in_=bqk[1])
        nc.gpsimd.dma_start(out=w_sb[2][:], in_=wqkv[2])
        bvb8_sb = const.tile([PD, 8 * PD], F32, tag="bvb8")
        nc.gpsimd.dma_start(out=bvb8_sb[:], in_=bvb8)
        for g in range(1, 4):
            nc.sync.dma_start(out=xtb_sb[:, 0, g * QC:(g + 1) * QC],
                              in_=xtb[0, :, g * QC:(g + 1) * QC])
        nc.gpsimd.dma_start(out=xtb_sb[:, 1, :], in_=xtb[1])
        wop_sb = const.tile([PD, E], BF16, tag="wop")
        for h in range(HPC):
            nc.gpsimd.dma_start(out=wop_sb[h * Dh:(h + 1) * Dh, :], in_=wo2[h])
        ones_sb = const.tile([1, Dh], BF16, tag="ones")
        rrT_sb = const.tile([32, 2 * QC], BF16, tag="rrT")

        # ---- warmup while the first DMAs are in flight ----
        # 8 dummy matmuls (~3.5us of continuous PE streaming) push the PE
        # p-state/HAM gate to full clock before phase 0; one dummy exp
        # preloads the ACT exp table (~1.3us ACT_TABLE_LOAD off the
        # critical path).  Dummies ride the scs ring in pairs (parity).
        warm = const.tile([PD, QC], BF16, tag="warm")
        nc.vector.memset(warm[:], 0.0)
        warmo = const.tile([PD, QC], BF16, tag="warmo")
        nc.scalar.activation(warmo[:], warm[:], EXP)
        for _ in range(8):
            wps = ps.tile([PD, 2 * QC], F32, tag="scs", name="warmps")
            nc.tensor.matmul(wps[:Dh, 0:QC], lhsT=warm[:, 0:Dh], rhs=warm[:])

        # ---- pipeline state ----
        phases = [(b, c) for b in range(B) for c in range(NQC)]
        NP = len(phases)
        slabs = {}   # (pi, kt) -> slab tile
        oas = {}     # phase idx -> oa PSUM tile [128, 1024] (rows 0:65 used)
        qts = {}     # batch -> qt tile
        kts = {}     # batch -> kt tile
        vaugs = {}   # batch -> vaug tile

        def emit_qkv_alloc(b):
            qt = perb.tile([PD, S], BF16, tag="qt", name=f"qt{b}")
            kt_t = perb.tile([PD, S], BF16, tag="kt", name=f"kt{b}")
            vaug = perb.tile([PD, NKT, HPC, VW], BF16, tag="vaug",
                             name=f"vaug{b}")
            # ones at col 64 (denom row 64: 32-aligned for the tail ACT ln)
            # and col 95 (denom row 95 = block-row 31 for the transpose path)
            nc.vector.memset(vaug[:, :, :, Dh + 1:VW - 1], 0.0)
            nc.vector.memset(vaug[:, :, :, Dh], 1.0)
            nc.vector.memset(vaug[:, :, :, VW - 1], 1.0)
            qts[b], kts[b], vaugs[b] = qt, kt_t, vaug

        # QKV projection pieces.  Mid-phase pieces ride the scs PSUM ring,
        # which the scores/exp ping-pong also uses; every piece consumes an
        # EVEN number of ring slots (real + dummy, or two real) so scores
        # parity is preserved -- an odd-slot piece makes the next scores
        # matmul WAR-wait on a 1-tile-old exp (~0.7us ACT stall each).
        def qkv_quarter(b, kind, g, p=None, pad=True):
            """One 512-wide Q (kind 0) or K (kind 1) quarter for batch b."""
            if p is None:
                p = ps.tile([PD, 2 * QC], F32, tag="scs", name="qkvps")
                if pad:
                    ps.tile([PD, 2 * QC], F32, tag="scs", name="par")
            dst = qts[b] if kind == 0 else kts[b]
            bias = bq_sb if kind == 0 else bk_sb
            sl_ = slice(g * QC, (g + 1) * QC)
            nc.tensor.matmul(p[:, 0:QC], lhsT=w_sb[kind][:],
                             rhs=xtb_sb[:, b, sl_])
            nc.vector.tensor_scalar_add(dst[:, sl_], p[:, 0:QC], bias[:])

        def qkv_half(b, kind, g, p=None):
            """One 1024-wide Q/K half (kind 0/1) or V half (kind 2),
            split over two ring slots (parity-neutral)."""
            if kind < 2:
                dst = qts[b] if kind == 0 else kts[b]
                bias = bq_sb if kind == 0 else bk_sb
                for j in range(2):
                    pj = p if p is not None else ps.tile(
                        [PD, 2 * QC], F32, tag="scs", name="qkvps")
                    sl_ = slice((2 * g + j) * QC, (2 * g + j + 1) * QC)
                    nc.tensor.matmul(pj[:, j * QC:(j + 1) * QC],
                                     lhsT=w_sb[kind][:],
                                     rhs=xtb_sb[:, b, sl_])
                    nc.vector.tensor_scalar_add(
                        dst[:, sl_], pj[:, j * QC:(j + 1) * QC], bias[:])
            else:
                for j in range(2):
                    pj = p if p is not None else ps.tile(
                        [PD, 2 * QC], F32, tag="scs", name="qkvps")
                    v_quad(b, 2 * g + j, pj, j)

        def v_quad(b, q, p, half):
            """V projection for k-tiles 4q..4q+3 into half `half` of p."""
            o = half * 4 * PD
            for i in range(4):
                st = 4 * q + i
                nc.tensor.matmul(p[:, o + i * PD:o + (i + 1) * PD],
                                 lhsT=xtb_sb[:, b, st * KT:(st + 1) * KT],
                                 rhs=w_sb[2][:])
            nc.vector.tensor_add(
                vaugs[b][:, 4 * q:4 * (q + 1), :, 0:Dh],
                p[:, o:o + 4 * PD].rearrange("p (t h d) -> p t h d", t=4, h=HPC),
                bvb8_sb[:, 0:4 * PD].rearrange("p (t h d) -> p t h d", t=4, h=HPC),
            )

        def v_eighth(b, e):
            """V projection for k-tiles 4e..4e+3, split over two ring slots
            (2 matmuls + drain each) to stay parity-neutral."""
            for half in range(2):
                p = ps.tile([PD, 2 * QC], F32, tag="scs", name="qkvps")
                o = half * 2
                for i in range(2):
                    st = 4 * e + o + i
                    nc.tensor.matmul(p[:, i * PD:(i + 1) * PD],
                                     lhsT=xtb_sb[:, b, st * KT:(st + 1) * KT],
                                     rhs=w_sb[2][:])
                nc.vector.tensor_add(
                    vaugs[b][:, 4 * e + o:4 * e + o + 2, :, 0:Dh],
                    p[:, 0:2 * PD].rearrange("p (t h d) -> p t h d", t=2, h=HPC),
                    bvb8_sb[:, 0:2 * PD].rearrange("p (t h d) -> p t h d",
                                                   t=2, h=HPC),
                )

        def emit_pv(pi, kt):
            oa = oas[pi]
            sl = slabs.pop((pi, kt))
            for h in range(HPC):
                nc.tensor.matmul(
                    oa[0:VW, h * QC:(h + 1) * QC],
                    lhsT=vaugs[phases[pi][0]][:, kt, h, :],
                    rhs=sl[:, h * QC:(h + 1) * QC],
                    start=(kt == 0), stop=(kt == NKT - 1),
                )

        def emit_recip(pi, mode=None):
            """1/denom for phase pi: oa row 64 -> rr [1 (or 32), 1024] bf16."""
            oa = oas[pi]
            if (mode or RECIP_MODE) == "trans":
                # 32x32 block transpose of oa rows 33:65 (all PV-written)
                # scatters the denom row (block-row 31) across partitions:
                # dnmT[i, 32b+31] = denom[32b+i].  Reciprocal runs on that
                # stride-32 view (free-size 32, ~0.2us); transposing back
                # packs row 31 = 1/denom.  rrT_sb is memset once and only
                # its 31-columns are rewritten each phase, keeping the
                # transpose-back input fully initialized.
                dnmT = normp.tile([32, 2 * QC], F32, tag="dnmT", name="dnmT")
                nc.vector.transpose(dnmT[:], oa[Dh:Dh + 32, :])
                # write 1/denom into c-position 0 of each block so the
                # transpose back packs it into row 0 (matmul rhs needs a
                # 32-aligned start partition)
                with nc.allow_low_precision(reason="bf16 1/denom"):
                    nc.vector.reciprocal(
                        rrT_sb[:].rearrange("p (b c) -> p b c", c=32)[:, :, 0],
                        dnmT[:].rearrange("p (b c) -> p b c", c=32)[:, :, 31],
                    )
                rr = normp.tile([32, 2 * QC], BF16, tag="rr", name="rr")
                nc.vector.transpose(rr[:], rrT_sb[:])
                return rr, 0
            rr = normp.tile([1, 2 * QC], BF16, tag="rr", name="rr")
            lnd = normp.tile([1, 2 * QC], F32, tag="lnd", name="lnd")
            nc.scalar.activation(lnd[:], oa[Dh:Dh + 1, :], LN)
            nc.scalar.activation(rr[:], lnd[:], EXP, scale=-1.0)
            return rr, 0

        def emit_bcast(pi, rrow):
            """Broadcast 1/denom across 64 partitions into oa rows 64:128."""
            oa = oas[pi]
            oa64 = oa[Dh:Dh + Dh, :]
            rr, row = rrow
            for h in range(HPC):
                nc.tensor.matmul(oa64[:, h * QC:(h + 1) * QC],
                                 lhsT=ones_sb[:],
                                 rhs=rr[row:row + 1, h * QC:(h + 1) * QC],
                                 tile_position=(0, Dh))

        def emit_otp(pi):
            """Per-head normalized output: otp [128, 512] bf16 (h1 -> rows 64+)."""
            oa = oas[pi]
            oa64 = oa[Dh:Dh + Dh, :]
            bc = normp.tile([Dh, 2 * QC], F32, tag="bc", name="bc")
            nc.vector.tensor_copy(bc[:], oa64[:])
            otp = normp.tile([PD, QC], BF16, tag="otp", name="otp")
            for h in range(HPC):
                nc.vector.tensor_mul(otp[h * Dh:(h + 1) * Dh, :],
                                     oa[0:Dh, h * QC:(h + 1) * QC],
                                     bc[:, h * QC:(h + 1) * QC])
            return otp

        def emit_outproj_pair(pi, otp, i, outsb, slices=None):
            """Out-projection pair #i (stile i//2, echunk i%2) for phase pi."""
            oa = oas[pi]
            st, ec = i // 2, i % 2
            esl = slice(ec * QC, (ec + 1) * QC)
            if slices is None:
                sl_ = oa[:, ec * QC:(ec + 1) * QC]
            else:
                sl_ = slices[i % len(slices)]
            nc.tensor.matmul(sl_, lhsT=otp[:, st * KT:(st + 1) * KT],
                             rhs=wop_sb[:, esl])
            if slices is not None and i % 2 == 1:
                # tail only: ACT is idle after the last exp; split staging
                nc.scalar.copy(outsb[:, esl], sl_)
            else:
                nc.vector.tensor_copy(outsb[:, esl], sl_)

        def flush(fp, slot, state):
            """Flush work for phase fp, scheduled at kt position `slot`
            of the following phase (or compressed in the tail)."""
            if fp < 0:
                return
            if slot == 3:
                state["rr"] = emit_recip(fp)
            elif slot == 4:
                emit_bcast(fp, state["rr"])
            elif slot == 5:
                state["ots"] = emit_otp(fp)
            elif 6 <= slot < 14:
                i = slot - 6
                b2, c2 = phases[fp]
                st, ec = i // 2, i % 2
                if ec == 0:
                    state["outsb"] = outp.tile([KT, E], BF16, tag="outsb",
                                               name="outsb")
                emit_outproj_pair(fp, state["ots"], i, state["outsb"],
                                  slices=state.get("slices"))
                ssl = slice(c2 * QC + st * KT, c2 * QC + (st + 1) * KT)
                if state.get("slices") is not None:
                    # tail: full-stile DMA (2KB rows) once both echunk
                    # copies land, rotated over three queues
                    if ec == 1:
                        eng = (nc.sync, nc.gpsimd, nc.scalar)[st % 3]
                        eng.dma_start(out=out[b2, ssl, :],
                                      in_=state["outsb"][:])
                elif ec == 1:
                    eng = nc.sync if st % 2 == 0 else nc.gpsimd
                    eng.dma_start(out=out[b2, ssl, :], in_=state["outsb"][:])

        # QKV extras: (pi, kt) -> emit fn.  Phase 0 builds batch 0 piecewise;
        # batch 1 parts ride retired PSUM slots in phases (0,1)-(0,3).
        extras = {
            (0, 0): lambda: v_eighth(0, 0),
            (0, 1): lambda: qkv_quarter(0, 1, 1),
            (0, 2): lambda: v_eighth(0, 1),
            (0, 3): lambda: qkv_quarter(0, 1, 2),
            (0, 4): lambda: v_eighth(0, 2),
            (0, 5): lambda: qkv_quarter(0, 1, 3),
            (0, 6): lambda: v_eighth(0, 3),
            (0, 8): lambda: qkv_quarter(0, 0, 1),
            (0, 10): lambda: qkv_quarter(0, 0, 2),
            (0, 12): lambda: qkv_quarter(0, 0, 3),
            (1, 14): lambda: qkv_half(1, 1, 0),
            (1, 15): lambda: qkv_half(1, 0, 0),
            (2, 14): lambda: qkv_half(1, 1, 1, p=oas[1]),
            (2, 15): lambda: qkv_half(1, 0, 1, p=oas[1]),
            (3, 14): lambda: qkv_half(1, 2, 0, p=oas[2]),
            (3, 15): lambda: qkv_half(1, 2, 1, p=oas[2]),
        }

        state = {}
        emit_qkv_alloc(0)
        emit_qkv_alloc(1)
        qkv_quarter(0, 0, 0, pad=False)   # Q chunk 0
        qkv_quarter(0, 1, 0, pad=False)   # K k-tiles 0-3
        for pi, (b, c) in enumerate(phases):
            qt, kt_t = qts[b], kts[b]
            csl = slice(c * QC, (c + 1) * QC)
            oas[pi] = ps.tile([PD, 2 * QC], F32, tag="oa", name=f"oa{pi}")
            for kt in range(NKT):
                scs = ps.tile([PD, 2 * QC], F32, tag="scs", name="scs")
                for h in range(HPC):
                    hsl = slice(Dh * h, Dh * (h + 1))
                    # 2x row tiling: both heads stream concurrently
                    nc.tensor.matmul(
                        scs[:, h * QC:(h + 1) * QC],
                        lhsT=kt_t[hsl, kt * KT:(kt + 1) * KT],
                        rhs=qt[hsl, csl],
                        tile_position=(Dh * h, 0),
                    )
                sl_t = slabp.tile([PD, 2 * QC], BF16, tag="slab", name="slab")
                nc.scalar.activation(sl_t[:], scs[:], EXP, scale=0.125)
                slabs[(pi, kt)] = sl_t
                # PV: 2 k-tiles behind exp; first two slots finish pi-1
                if kt >= 2:
                    emit_pv(pi, kt - 2)
                elif pi >= 1:
                    emit_pv(pi - 1, NKT - 2 + kt)
                if pi >= 1:
                    flush(pi - 1, kt, state)
                ex = extras.get((pi, kt))
                if ex is not None:
                    ex()

        # ---- tail: finish PV for the last phase, flush it compressed ----
        last = NP - 1
        emit_pv(last, NKT - 2)
        emit_pv(last, NKT - 1)
        # ACT chain here: shorter serial latency than the two stream
        # transposes, and ACT is idle once the last exp retires
        state["rr"] = emit_recip(last, mode="act")
        emit_bcast(last, state["rr"])
        state["ots"] = emit_otp(last)
        # rotate through FOUR retired PSUM slices (oa last + last-1) so the
        # matmul->copy->DMA chain pipelines 4 deep
        state["slices"] = [
            oas[last][:, 0:QC], oas[last][:, QC:2 * QC],
            oas[last - 1][:, 0:QC], oas[last - 1][:, QC:2 * QC],
        ]
        for slot in range(6, 14):
            flush(last, slot, state)

    from concourse.library_overlay import lower_extended_insts

    lower_extended_insts(nc)
    split_multi_waits(nc)
    return nc


def prep_core_inputs(c, x, Wq, Wk, Wv, bq, bk, bv, Wo):
    h0, h1 = HPC * c, HPC * c + 1
    xT_c = np.ascontiguousarray(
        np.transpose(x[:, :, c * PD:(c + 1) * PD], (0, 2, 1))
    ).astype(BF)
    wqkv = np.zeros((3, PD, PD), np.float32)
    for i, W in enumerate((Wq, Wk, Wv)):
        wqkv[i, :Dh, :Dh] = W[h0]
        wqkv[i, Dh:, Dh:] = W[h1]
    bqk = np.stack([
        np.concatenate([bq[h0], bq[h1]])[:, None],
        np.concatenate([bk[h0], bk[h1]])[:, None],
    ]).astype(np.float32)
    bv_pair = np.concatenate([bv[h0], bv[h1]])          # [128]
    bvb8 = np.tile(bv_pair[None, :], (PD, 8)).astype(np.float32)
    wo2 = np.stack([Wo[h0 * Dh:(h0 + 1) * Dh], Wo[h1 * Dh:(h1 + 1) * Dh]])
    return {
        "xtb": xT_c,
        "wqkv": wqkv.astype(BF),
        "bqk": bqk,
        "bvb8": bvb8,
        "wo2": wo2.astype(BF),
    }


_CACHE = {}


def _get_nc():
    if "nc" not in _CACHE:
        _CACHE["nc"] = build_program()
    return _CACHE["nc"]


def kernel(x, Wq, Wk, Wv, bq, bk, bv, Wo, bo, _trace=False, _trace_kwargs=None):
    x, Wq, Wk, Wv, bq, bk, bv, Wo, bo = (
        np.asarray(a, np.float32) for a in (x, Wq, Wk, Wv, bq, bk, bv, Wo, bo)
    )
    nc = _get_nc()
    in_maps = [
        prep_core_inputs(c, x, Wq, Wk, Wv, bq, bk, bv, Wo) for c in range(NCORES)
    ]
    res = run_bass_kernel_spmd(
        nc, in_maps, list(range(NCORES)), trace=_trace, **(_trace_kwargs or {})
    )
    acc = np.asarray(res.results[0]["out"], np.float32)
    for c in range(1, NCORES):
        acc = acc + np.asarray(res.results[c]["out"], np.float32)
    acc += bo[None, None, :]
    if _trace:
        _CACHE["last_results"] = res
    return acc


# revision 22
# speedup vs baseline: 1.2661x; 1.0109x over previous
"""Head-parallel multi-head attention on 8 Trainium2 NeuronCores (v3).

Sharding: 2 heads per core (head axis split across 8 cores). Each core
computes its heads' Q/K/V projections (block-diagonal 128x128 weights,
both heads packed on the partition axis), full attention for its 2
heads, and a per-head partial W_o projection over its 128 head-dims.
The host sums the 8 partial outputs (the all-gather + W_o is
algebraically a sum of per-core partial matmuls) and adds b_o.

v3 over v2 (~229us): kt-lag pipeline instead of phase-lag.
  * PV(p, kt) runs 2 k-tiles behind exp(p, kt) inside the SAME phase
    (v2 lagged a full phase), and normalize/outproj/DMA for phase p
    flush during phase p+1 (v2: p+2).  The drain tail after the last
    exp shrinks from ~42us to a few us.
  * startup: x is DMA'd in 512-col chunks and Q/K are produced in
    512-col quarter matmuls, so the first scores matmul only waits for
    one small DMA chain; first exp fires ~3us in (v2: ~16us).
  * softmax 1/denom comes from a single DVE reciprocal (bf16 out)
    instead of the two-ACTIVATE exp(-ln d) chain: ACT runs only the
    128 [128,1024] exps (its hard floor, ~1.11us each).
  * per-phase PSUM stays at exactly 8 banks: scores ping-pong
    2x[128,1024] + oa(p)/oa(p-1) 2x[128,1024]; outproj(p-1) writes
    into retired oa(p-1) slices, late QKV parts reuse retired banks.
"""

import os
import sys
from contextlib import ExitStack

import numpy as np

for _p in ("/opt/trn_rl_repo", os.path.expanduser("~/.axon_site/_ro/trn_rl_repo")):
    if os.path.isdir(_p) and _p not in sys.path:
        sys.path.append(_p)

import ml_dtypes

import concourse.bass as bass
import concourse.tile as tile
from concourse import mybir
from concourse.bass_utils import run_bass_kernel_spmd

B, S, E, H = 2, 2048, 1024, 16
Dh = E // H           # 64
NCORES = 8
HPC = H // NCORES     # 2 heads per core
PD = HPC * Dh         # 128 pair dims per core
QC = 512              # q-chunk width
NQC = S // QC         # 4
KT = 128              # k-tile rows
VW = 96               # vaug width: v dims 0:64, zeros, denom-ones col at 95
NKT = S // KT         # 16
F32 = mybir.dt.float32
BF16 = mybir.dt.bfloat16
EXP = mybir.ActivationFunctionType.Exp
LN = mybir.ActivationFunctionType.Ln
BF = ml_dtypes.bfloat16

# 1/denom strategies.  Measured: a [1,1024] InstReciprocal is ~8 cyc/elem
# (6.55us) and head-of-line-blocks the DVE queue -> never use it wide.
# TRANS: DVE 32x32 stream-transpose the denom row into a [32,32]-strided
# layout, reciprocal at free-size 32 (~0.2us), transpose back.  Fallback:
# ACT exp(-ln d) chain (2.2us of ACT per phase).
RECIP_MODE = "trans"  # "trans" | "act"


def split_multi_waits(nc):
    """Split multi-wait instructions into chained single-wait EventSemaphores.

    The walrus build here accepts at most ONE sync-wait command per
    instruction, while Tile emits several. Rewrite each instruction with
    N>1 waits into (N-1) same-engine EventSemaphore instructions (one
    wait each) followed by the instruction keeping its last wait --
    per-engine program order makes this equivalent.
    """
    n_split = 0
    for f in nc.m.functions:
        for blk in f.blocks:
            insts = list(blk.instructions)
            new = []
            for inst in insts:
                si = inst.sync_info
                waits = list(si.on_wait) if si is not None and si.on_wait else []
                if len(waits) > 1:
                    for j, w in enumerate(waits[:-1]):
                        ev = mybir.InstEventSemaphore(
                            name=f"{inst.name}-wsplit{j}", ins=[], outs=[]
                        )
                        ev.engine = inst.engine
                        ev.sync_info = mybir.SyncInfo(on_wait=[w], on_update=[])
                        nc.register_instruction(ev, overwrite=True)
                        new.append(ev)
                    si.on_wait = waits[-1:]
                    n_split += 1
                new.append(inst)
            blk.instructions = new
    return n_split


def build_program():
    nc = bass.Bass("TRN2", target_bir_lowering=False, debug=False)

    xtb = nc.dram_tensor("xtb", [B, PD, S], BF16, kind="ExternalInput").ap()
    wqkv = nc.dram_tensor("wqkv", [3, PD, PD], BF16, kind="ExternalInput").ap()
    bqk = nc.dram_tensor("bqk", [2, PD, 1], F32, kind="ExternalInput").ap()
    bvb8 = nc.dram_tensor("bvb8", [PD, 8 * PD], F32, kind="ExternalInput").ap()
    wo2 = nc.dram_tensor("wo2", [HPC, Dh, E], BF16, kind="ExternalInput").ap()
    out = nc.dram_tensor("out", [B, S, E], BF16, kind="ExternalOutput").ap()

    with tile.TileContext(nc) as tc, ExitStack() as ctx:
        const = ctx.enter_context(tc.tile_pool(name="const", bufs=1))
        perb = ctx.enter_context(tc.tile_pool(name="perb", bufs=2))
        slabp = ctx.enter_context(tc.tile_pool(name="slab", bufs=6))
        normp = ctx.enter_context(tc.tile_pool(name="norm", bufs=2))
        outp = ctx.enter_context(tc.tile_pool(name="outp", bufs=4))
        ps = ctx.enter_context(tc.tile_pool(name="ps", bufs=2, space="PSUM"))

        # ---- constants: order so the first Q/K quarter's deps land first ----
        w_sb = []
        for i in range(3):
            w_sb.append(const.tile([PD, PD], BF16, tag=f"w{i}", name=f"w{i}"))
        nc.gpsimd.dma_start(out=w_sb[0][:], in_=wqkv[0])
        nc.gpsimd.dma_start(out=w_sb[1][:], in_=wqkv[1])
        xtb_sb = const.tile([PD, B, S], BF16)
        nc.sync.dma_start(out=xtb_sb[:, 0, 0:QC], in_=xtb[0, :, 0:QC])
        bq_sb = const.tile([PD, 1], F32, tag="bq")
        nc.sync.dma_start(out=bq_sb[:], in_=bqk[0])
        bk_sb = const.tile([PD, 1], F32, tag="bk")
        nc.sync.dma_start(out=bk_sb[:], in_=bqk[1])
        nc.gpsimd.dma_start(out=w_sb[2][:], in_=wqkv[2])
        bvb8_sb = const.tile([PD, 8 * PD], F32, tag="bvb8")
        nc.gpsimd.dma_start(out=bvb8_sb[:], in_=bvb8)
        for g in range(1, 4):
            nc.sync.dma_start(out=xtb_sb[:, 0, g * QC:(g + 1) * QC],
                              in_=xtb[0, :, g * QC:(g + 1) * QC])
        nc.gpsimd.dma_start(out=xtb_sb[:, 1, :], in_=xtb[1])
        wop_sb = const.tile([PD, E], BF16, tag="wop")
        for h in range(HPC):
            nc.gpsimd.dma_start(out=wop_sb[h * Dh:(h + 1) * Dh, :], in_=wo2[h])
        ones_sb = const.tile([1, Dh], BF16, tag="ones")
        rrT_sb = const.tile([32, 2 * QC], BF16, tag="rrT")

        # ---- warmup while the first DMAs are in flight ----
        # 8 dummy matmuls (~3.5us of continuous PE streaming) push the PE
        # p-state/HAM gate to full clock before phase 0; one dummy exp
        # preloads the ACT exp table (~1.3us ACT_TABLE_LOAD off the
        # critical path).  Dummies ride the scs ring in pairs (parity).
        warm = const.tile([PD, QC], BF16, tag="warm")
        nc.vector.memset(warm[:], 0.0)
        warmo = const.tile([PD, QC], BF16, tag="warmo")
        nc.scalar.activation(warmo[:], warm[:], EXP)
        for _ in range(8):
            wps = ps.tile([PD, 2 * QC], F32, tag="scs", name="warmps")
            nc.tensor.matmul(wps[:Dh, 0:QC], lhsT=warm[:, 0:Dh], rhs=warm[:])

        # ---- pipeline state ----
        phases = [(b, c) for b in range(B) for c in range(NQC)]
        NP = len(phases)
        slabs = {}   # (pi, kt) -> slab tile
        oas = {}     # phase idx -> oa PSUM tile [128, 1024] (rows 0:65 used)
        qts = {}     # batch -> qt tile
        kts = {}     # batch -> kt tile
        vaugs = {}   # batch -> vaug tile

        def emit_qkv_alloc(b):
            qt = perb.tile([PD, S], BF16, tag="qt", name=f"qt{b}")
            kt_t = perb.tile([PD, S], BF16, tag="kt", name=f"kt{b}")
            vaug = perb.tile([PD, NKT, HPC, VW], BF16, tag="vaug",
                             name=f"vaug{b}")
            qts[b], kts[b], vaugs[b] = qt, kt_t, vaug

        def emit_vaug_memset(b):
            # ones at col 64 (denom row 64: 32-aligned for the tail ACT ln)
            # and col 95 (denom row 95 = block-row 31 for the transpose path)
            vaug = vaugs[b]
            nc.vector.memset(vaug[:, :, :, Dh + 1:VW - 1], 0.0)
            nc.vector.memset(vaug[:, :, :, Dh], 1.0)
            nc.vector.memset(vaug[:, :, :, VW - 1], 1.0)

        # QKV projection pieces.  Mid-phase pieces ride the scs PSUM ring,
        # which the scores/exp ping-pong also uses; every piece consumes an
        # EVEN number of ring slots (real + dummy, or two real) so scores
        # parity is preserved -- an odd-slot piece makes the next scores
        # matmul WAR-wait on a 1-tile-old exp (~0.7us ACT stall each).
        def qkv_quarter(b, kind, g, p=None, pad=True):
            """One 512-wide Q (kind 0) or K (kind 1) quarter for batch b."""
            if p is None:
                p = ps.tile([PD, 2 * QC], F32, tag="scs", name="qkvps")
                if pad:
                    ps.tile([PD, 2 * QC], F32, tag="scs", name="par")
            dst = qts[b] if kind == 0 else kts[b]
            bias = bq_sb if kind == 0 else bk_sb
            sl_ = slice(g * QC, (g + 1) * QC)
            nc.tensor.matmul(p[:, 0:QC], lhsT=w_sb[kind][:],
                             rhs=xtb_sb[:, b, sl_])
            nc.vector.tensor_scalar_add(dst[:, sl_], p[:, 0:QC], bias[:])

        def qkv_half(b, kind, g, p=None):
            """One 1024-wide Q/K half (kind 0/1) or V half (kind 2),
            split over two ring slots (parity-neutral)."""
            if kind < 2:
                dst = qts[b] if kind == 0 else kts[b]
                bias = bq_sb if kind == 0 else bk_sb
                for j in range(2):
                    pj = p if p is not None else ps.tile(
                        [PD, 2 * QC], F32, tag="scs", name="qkvps")
                    sl_ = slice((2 * g + j) * QC, (2 * g + j + 1) * QC)
                    nc.tensor.matmul(pj[:, j * QC:(j + 1) * QC],
                                     lhsT=w_sb[kind][:],
                                     rhs=xtb_sb[:, b, sl_])
                    nc.vector.tensor_scalar_add(
                        dst[:, sl_], pj[:, j * QC:(j + 1) * QC], bias[:])
            else:
                for j in range(2):
                    pj = p if p is not None else ps.tile(
                        [PD, 2 * QC], F32, tag="scs", name="qkvps")
                    v_quad(b, 2 * g + j, pj, j)

        def v_quad(b, q, p, half):
            """V projection for k-tiles 4q..4q+3 into half `half` of p."""
            o = half * 4 * PD
            for i in range(4):
                st = 4 * q + i
                nc.tensor.matmul(p[:, o + i * PD:o + (i + 1) * PD],
                                 lhsT=xtb_sb[:, b, st * KT:(st + 1) * KT],
                                 rhs=w_sb[2][:])
            nc.vector.tensor_add(
                vaugs[b][:, 4 * q:4 * (q + 1), :, 0:Dh],
                p[:, o:o + 4 * PD].rearrange("p (t h d) -> p t h d", t=4, h=HPC),
                bvb8_sb[:, 0:4 * PD].rearrange("p (t h d) -> p t h d", t=4, h=HPC),
            )

        def v_eighth(b, e):
            """V projection for k-tiles 4e..4e+3, split over two ring slots
            (2 matmuls + drain each) to stay parity-neutral."""
            for half in range(2):
                p = ps.tile([PD, 2 * QC], F32, tag="scs", name="qkvps")
                o = half * 2
                for i in range(2):
                    st = 4 * e + o + i
                    nc.tensor.matmul(p[:, i * PD:(i + 1) * PD],
                                     lhsT=xtb_sb[:, b, st * KT:(st + 1) * KT],
                                     rhs=w_sb[2][:])
                nc.vector.tensor_add(
                    vaugs[b][:, 4 * e + o:4 * e + o + 2, :, 0:Dh],
                    p[:, 0:2 * PD].rearrange("p (t h d) -> p t h d", t=2, h=HPC),
                    bvb8_sb[:, 0:2 * PD].rearrange("p (t h d) -> p t h d",
                                                   t=2, h=HPC),
                )

        def emit_pv(pi, kt):
            oa = oas[pi]
            sl = slabs.pop((pi, kt))
            for h in range(HPC):
                nc.tensor.matmul(
                    oa[0:VW, h * QC:(h + 1) * QC],
                    lhsT=vaugs[phases[pi][0]][:, kt, h, :],
                    rhs=sl[:, h * QC:(h + 1) * QC],
                    start=(kt == 0), stop=(kt == NKT - 1),
                )

        def emit_recip(pi, mode=None):
            """1/denom for phase pi: oa row 64 -> rr [1 (or 32), 1024] bf16."""
            oa = oas[pi]
            if (mode or RECIP_MODE) == "trans":
                # 32x32 block transpose of oa rows 33:65 (all PV-written)
                # scatters the denom row (block-row 31) across partitions:
                # dnmT[i, 32b+31] = denom[32b+i].  Reciprocal runs on that
                # stride-32 view (free-size 32, ~0.2us); transposing back
                # packs row 31 = 1/denom.  rrT_sb is memset once and only
                # its 31-columns are rewritten each phase, keeping the
                # transpose-back input fully initialized.
                dnmT = normp.tile([32, 2 * QC], F32, tag="dnmT", name="dnmT")
                nc.vector.transpose(dnmT[:], oa[Dh:Dh + 32, :])
                # write 1/denom into c-position 0 of each block so the
                # transpose back packs it into row 0 (matmul rhs needs a
                # 32-aligned start partition)
                with nc.allow_low_precision(reason="bf16 1/denom"):
                    nc.vector.reciprocal(
                        rrT_sb[:].rearrange("p (b c) -> p b c", c=32)[:, :, 0],
                        dnmT[:].rearrange("p (b c) -> p b c", c=32)[:, :, 31],
                    )
                rr = normp.tile([32, 2 * QC], BF16, tag="rr", name="rr")
                nc.vector.transpose(rr[:], rrT_sb[:])
                return rr, 0
            rr = normp.tile([1, 2 * QC], BF16, tag="rr", name="rr")
            lnd = normp.tile([1, 2 * QC], F32, tag="lnd", name="lnd")
            nc.scalar.activation(lnd[:], oa[Dh:Dh + 1, :], LN)
            nc.scalar.activation(rr[:], lnd[:], EXP, scale=-1.0)
            return rr, 0

        def emit_bcast(pi, rrow):
            """Broadcast 1/denom across 64 partitions into oa rows 64:128."""
            oa = oas[pi]
            oa64 = oa[Dh:Dh + Dh, :]
            rr, row = rrow
            for h in range(HPC):
                nc.tensor.matmul(oa64[:, h * QC:(h + 1) * QC],
                                 lhsT=ones_sb[:],
                                 rhs=rr[row:row + 1, h * QC:(h + 1) * QC],
                                 tile_position=(0, Dh))

        def emit_otp(pi):
            """Per-head normalized output: otp [128, 512] bf16 (h1 -> rows 64+)."""
            oa = oas[pi]
            oa64 = oa[Dh:Dh + Dh, :]
            bc = normp.tile([Dh, 2 * QC], F32, tag="bc", name="bc")
            nc.vector.tensor_copy(bc[:], oa64[:])
            otp = normp.tile([PD, QC], BF16, tag="otp", name="otp")
            for h in range(HPC):
                nc.vector.tensor_mul(otp[h * Dh:(h + 1) * Dh, :],
                                     oa[0:Dh, h * QC:(h + 1) * QC],
                                     bc[:, h * QC:(h + 1) * QC])
            return otp

        def emit_outproj_pair(pi, otp, i, outsb, slices=None):
            """Out-projection pair #i (stile i//2, echunk i%2) for phase pi."""
            oa = oas[pi]
            st, ec = i // 2, i % 2
            esl = slice(ec * QC, (ec + 1) * QC)
            if slices is None:
                sl_ = oa[:, ec * QC:(ec + 1) * QC]
            else:
                sl_ = slices[i % len(slices)]
            nc.tensor.matmul(sl_, lhsT=otp[:, st * KT:(st + 1) * KT],
                             rhs=wop_sb[:, esl])
            if slices is not None and i % 2 == 1:
                # tail only: ACT is idle after the last exp; split staging
                nc.scalar.copy(outsb[:, esl], sl_)
            else:
                nc.vector.tensor_copy(outsb[:, esl], sl_)

        def flush(fp, slot, state):
            """Flush work for phase fp, scheduled at kt position `slot`
            of the following phase (or compressed in the tail)."""
            if fp < 0:
                return
            if slot == 3:
                state["rr"] = emit_recip(fp)
            elif slot == 4:
                emit_bcast(fp, state["rr"])
            elif slot == 5:
                state["ots"] = emit_otp(fp)
            elif 6 <= slot < 14:
                i = slot - 6
                b2, c2 = phases[fp]
                st, ec = i // 2, i % 2
                if ec == 0:
                    state["outsb"] = outp.tile([KT, E], BF16, tag="outsb",
                                               name="outsb")
                emit_outproj_pair(fp, state["ots"], i, state["outsb"],
                                  slices=state.get("slices"))
                ssl = slice(c2 * QC + st * KT, c2 * QC + (st + 1) * KT)
                if state.get("slices") is not None:
                    # tail: full-stile DMA (2KB rows) once both echunk
                    # copies land, rotated over three queues
                    if ec == 1:
                        eng = (nc.sync, nc.gpsimd, nc.scalar)[st % 3]
                        eng.dma_start(out=out[b2, ssl, :],
                                      in_=state["outsb"][:])
                elif ec == 1:
                    eng = nc.sync if st % 2 == 0 else nc.gpsimd
                    eng.dma_start(out=out[b2, ssl, :], in_=state["outsb"][:])

        # QKV extras: (pi, kt) -> emit fn.  Phase 0 builds batch 0 piecewise;
        # batch 1 parts ride retired PSUM slots in phases (0,1)-(0,3).
        extras = {
            (0, 0): lambda: v_eighth(0, 0),
            (0, 1): lambda: qkv_quarter(0, 1, 1),
            (0, 2): lambda: v_eighth(0, 1),
            (0, 3): lambda: qkv_quarter(0, 1, 2),
            (0, 4): lambda: v_eighth(0, 2),
            (0, 5): lambda: qkv_quarter(0, 1, 3),
            (0, 6): lambda: v_eighth(0, 3),
            (0, 8): lambda: qkv_quarter(0, 0, 1),
            (0, 10): lambda: qkv_quarter(0, 0, 2),
            (0, 12): lambda: qkv_quarter(0, 0, 3),
            (1, 14): lambda: qkv_half(1, 1, 0),
            (1, 15): lambda: qkv_half(1, 0, 0),
            (2, 14): lambda: qkv_half(1, 1, 1, p=oas[1]),
            (2, 15): lambda: qkv_half(1, 0, 1, p=oas[1]),
            (3, 14): lambda: qkv_half(1, 2, 0, p=oas[2]),
            (3, 15): lambda: qkv_half(1, 2, 1, p=oas[2]),
        }

        state = {}
        emit_qkv_alloc(0)
        emit_qkv_alloc(1)
        qkv_quarter(0, 0, 0, pad=False)   # Q chunk 0
        qkv_quarter(0, 1, 0, pad=False)   # K k-tiles 0-3
        # memsets sit behind the lead-in drains on the in-order DVE queue
        nc.vector.memset(ones_sb[:], 1.0)
        nc.vector.memset(rrT_sb[:], 0.0)
        emit_vaug_memset(0)
        emit_vaug_memset(1)
        for pi, (b, c) in enumerate(phases):
            qt, kt_t = qts[b], kts[b]
            csl = slice(c * QC, (c + 1) * QC)
            oas[pi] = ps.tile([PD, 2 * QC], F32, tag="oa", name=f"oa{pi}")
            for kt in range(NKT):
                scs = ps.tile([PD, 2 * QC], F32, tag="scs", name="scs")
                for h in range(HPC):
                    hsl = slice(Dh * h, Dh * (h + 1))
                    # 2x row tiling: both heads stream concurrently
                    nc.tensor.matmul(
                        scs[:, h * QC:(h + 1) * QC],
                        lhsT=kt_t[hsl, kt * KT:(kt + 1) * KT],
                        rhs=qt[hsl, csl],
                        tile_position=(Dh * h, 0),
                    )
                sl_t = slabp.tile([PD, 2 * QC], BF16, tag="slab", name="slab")
                nc.scalar.activation(sl_t[:], scs[:], EXP, scale=0.125)
                slabs[(pi, kt)] = sl_t
                # PV: 2 k-tiles behind exp; first two slots finish pi-1
                if kt >= 2:
                    emit_pv(pi, kt - 2)
                elif pi >= 1:
                    emit_pv(pi - 1, NKT - 2 + kt)
                if pi >= 1:
                    flush(pi - 1, kt, state)
                ex = extras.get((pi, kt))
                if ex is not None:
                    ex()

        # ---- tail: finish PV for the last phase, flush it compressed ----
        last = NP - 1
        emit_pv(last, NKT - 2)
        emit_pv(last, NKT - 1)
        # ACT chain here: shorter serial latency than the two stream
        # transposes, and ACT is idle once the last exp retires
        state["rr"] = emit_recip(last, mode="act")
        emit_bcast(last, state["rr"])
        state["ots"] = emit_otp(last)
        # rotate through FOUR retired PSUM slices (oa last + last-1) so the
        # matmul->copy->DMA chain pipelines 4 deep
        state["slices"] = [
            oas[last][:, 0:QC], oas[last][:, QC:2 * QC],
            oas[last - 1][:, 0:QC], oas[last - 1][:, QC:2 * QC],
        ]
        for slot in range(6, 14):
            flush(last, slot, state)

    from concourse.library_overlay import lower_extended_insts

    lower_extended_insts(nc)
    split_multi_waits(nc)
    return nc


def prep_core_inputs(c, x, Wq, Wk, Wv, bq, bk, bv, Wo):
    h0, h1 = HPC * c, HPC * c + 1
    xT_c = np.ascontiguousarray(
        np.transpose(x[:, :, c * PD:(c + 1) * PD], (0, 2, 1))
    ).astype(BF)
    wqkv = np.zeros((3, PD, PD), np.float32)
    for i, W in enumerate((Wq, Wk, Wv)):
        wqkv[i, :Dh, :Dh] = W[h0]
        wqkv[i, Dh:, Dh:] = W[h1]
    bqk = np.stack([
        np.concatenate([bq[h0], bq[h1]])[:, None],
        np.concatenate([bk[h0], bk[h1]])[:, None],
    ]).astype(np.float32)
    bv_pair = np.concatenate([bv[h0], bv[h1]])          # [128]
    bvb8 = np.tile(bv_pair[None, :], (PD, 8)).astype(np.float32)
    wo2 = np.stack([Wo[h0 * Dh:(h0 + 1) * Dh], Wo[h1 * Dh:(h1 + 1) * Dh]])
    return {
        "xtb": xT_c,
        "wqkv": wqkv.astype(BF),
        "bqk": bqk,
        "bvb8": bvb8,
        "wo2": wo2.astype(BF),
    }


_CACHE = {}


def _get_nc():
    if "nc" not in _CACHE:
        _CACHE["nc"] = build_program()
    return _CACHE["nc"]


def kernel(x, Wq, Wk, Wv, bq, bk, bv, Wo, bo, _trace=False, _trace_kwargs=None):
    x, Wq, Wk, Wv, bq, bk, bv, Wo, bo = (
        np.asarray(a, np.float32) for a in (x, Wq, Wk, Wv, bq, bk, bv, Wo, bo)
    )
    nc = _get_nc()
    in_maps = [
        prep_core_inputs(c, x, Wq, Wk, Wv, bq, bk, bv, Wo) for c in range(NCORES)
    ]
    res = run_bass_kernel_spmd(
        nc, in_maps, list(range(NCORES)), trace=_trace, **(_trace_kwargs or {})
    )
    acc = np.asarray(res.results[0]["out"], np.float32)
    for c in range(1, NCORES):
        acc = acc + np.asarray(res.results[c]["out"], np.float32)
    acc += bo[None, None, :]
    if _trace:
        _CACHE["last_results"] = res
    return acc
